# revision 27
# baseline (speedup 1.0000x reference)
"""BeamVQ (VQ-VAE fwd) Trainium2 kernel — 8-core batch-parallel.

Strategy: shard batch 16 -> 8 cores x 2 images. Convs as shift-matmuls on PE
(channels on partitions, PSUM accumulation). Training-mode BN via per-channel
sum/sumsq with a tiny in-kernel AllReduce per BN layer (8 total). VQ: PE fp32
distance scores s = z.e - |e|^2/2, DVE max/max_index top-8, gpsimd ap_gather
codebook lookup. ConvTranspose via output-parity decomposition. Loss /
perplexity partial sums are finished on the host.

Precision: encoder + pre-VQ + distance matmuls in true fp32 (top-1 gaps go
down to 5e-5; fp32r's ~1e-5 operand rounding flips argmins and corrupts the
reconstruction by O(1)). Decoder matmuls run in fp32r (4x faster PE streams;
weights/activations are rounded to f32r by their ACT producers as the BIR
verifier requires) giving recon absmax ~1.4e-3 on scale 3.7; set DEC_DT = f32
to get absmax ~6e-6 at +~550us.

Measured (8 trn2 cores via axon/PJRT): recon absmax 1.38e-3 (rel 3.8e-4),
loss rel 3.5e-7, perplexity rel 5.5e-6 vs the fp32 CPU reference.
Cost-model timeline estimate ~1.36 ms end-to-end (excl. AllReduce latency).
"""
import numpy as np

import concourse.bass as bass
import concourse.mybir as mybir
from concourse import bacc, tile
from concourse.bass_utils import run_bass_kernel_spmd

f32 = mybir.dt.float32
f32r = mybir.dt.float32r
u16 = mybir.dt.uint16
i16 = mybir.dt.int16

NCORES = 8
ENC_DT = f32    # z-path matmul dtype (must stay fp32)
DEC_DT = f32r   # decoder matmul dtype (operands produced as rounded f32r)

POS9 = [(ky, kx) for ky in range(3) for kx in range(3)]
# transposed-conv k4 s2 p1 parity taps: parity -> [(k, shift)]
TAPS = {0: [(1, 0), (3, 1)], 1: [(0, -1), (2, 0)]}

A = mybir.ActivationFunctionType


def _mm(nc, ps, lhsT, rhs, dt, start, stop):
    # operands must already carry dtype `dt` (f32r data must be produced
    # rounded — the BIR verifier rejects plain bitcasts of f32 data)
    nc.tensor.matmul(ps, lhsT, rhs, start=start, stop=stop)


def _conv3x3(nc, pp, src, wsl, Cin, Cout, dt, emit, name):
    """src: padded-66 AP [>=Cin, 4356]; wsl(pos) -> lhsT [Cin, Cout];
    emit(bank, ps) consumes psum [Cout, 512] for out rows 8*bank..8*bank+8."""
    v = src.rearrange("p (h w) -> p h w", w=66)
    for bank in range(8):
        y0 = 8 * bank
        ps = pp.tile([Cout, 512], f32, tag="ps", name=f"{name}_ps{bank}")
        for pos, (ky, kx) in enumerate(POS9):
            rhs = v[0:Cin, y0 + ky:y0 + ky + 8, kx:kx + 64]
            _mm(nc, ps[:], wsl(pos), rhs, dt, pos == 0, pos == 8)
        emit(bank, ps)


def _memset_rings66(nc, t):
    v = t.bitcast(mybir.dt.uint32).rearrange("p (h w) -> p h w", w=66)
    nc.gpsimd.memset(v[:, 0:66:65, :], 0)
    nc.gpsimd.memset(v[:, :, 0:1], 0)
    nc.gpsimd.memset(v[:, :, 65:66], 0)


def _memset_rings130(nc, t):
    v = t.bitcast(mybir.dt.uint32).rearrange("p (h w) -> p h w", w=130)
    nc.gpsimd.memset(v[:, 0:130:129, :], 0)
    nc.gpsimd.memset(v[:, :, 0:1], 0)
    nc.gpsimd.memset(v[:, :, 128:130], 0)


def _build(with_cc=True):
    nc = bacc.Bacc("TRN2", target_bir_lowering=False, debug=False,
                   enable_asserts=False,
                   num_devices=NCORES if with_cc else 1)

    def din(name, shape, dt=f32):
        return nc.dram_tensor(name, shape, dt, kind="ExternalInput")

    def dout(name, shape, dt=f32):
        return nc.dram_tensor(name, shape, dt, kind="ExternalOutput")

    xcol_d = din("xcol", [16, 2 * 16384])
    w1_d = din("w1", [16, 128])
    b1_d = din("b1", [128, 1])
    w2p_d = din("w2p", [128, 8 * 128])
    b2_d = din("b2", [128, 1])
    w3_d = din("w3", [128, 9 * 128])
    b3_d = din("b3", [128, 1])
    ewa_d = din("ewa", [128, 2 * 9 * 32])
    ewb_d = din("ewb", [32, 2 * 128])
    eg1_d = din("eg1", [32, 2])
    eb1_d = din("eb1", [32, 2])
    eg2_d = din("eg2", [128, 2])
    eb2_d = din("eb2", [128, 2])
    prew_d = din("prew", [128, 128])
    preb_d = din("preb", [128, 1])
    embT_d = din("embT", [128, 1024])
    e2neg_d = din("e2neg", [1, 1024])
    dw1_d = din("dw1", [128, 9 * 128])
    db1_d = din("db1", [128, 1])
    dwa_d = din("dwa", [128, 2 * 9 * 32])
    dwb_d = din("dwb", [32, 2 * 128])
    dg1_d = din("dg1", [32, 2])
    db1r_d = din("db1r", [32, 2])
    dg2_d = din("dg2", [128, 2])
    db2r_d = din("db2r", [128, 2])
    dt1w_d = din("dt1w", [128, 16 * 128])
    dt1b_d = din("dt1b", [128, 1])
    dt2w_d = din("dt2w", [128, 24])

    outy_d = dout("out_y", [2, 4, 16384])
    osmax_d = dout("out_smax", [128, 64])
    oidx2_d = dout("out_idx2", [128, 64], u16)
    ozsq_d = dout("out_zsq", [128, 1])

    with tile.TileContext(nc) as tc:
        with tc.tile_pool(name="wpool", bufs=1) as wp, \
             tc.tile_pool(name="main", bufs=1) as mp, \
             tc.tile_pool(name="small", bufs=2) as sp, \
             tc.tile_pool(name="dram", bufs=1, space="DRAM") as dr:

            w1 = wp.tile_from(w1_d[:])
            b1 = wp.tile_from(b1_d[:])
            w2p = wp.tile_from(w2p_d[:])
            b2 = wp.tile_from(b2_d[:])
            w3 = wp.tile_from(w3_d[:])
            b3 = wp.tile_from(b3_d[:])
            ewa = wp.tile_from(ewa_d[:])
            ewb = wp.tile_from(ewb_d[:])
            eg1 = wp.tile_from(eg1_d[:])
            eb1 = wp.tile_from(eb1_d[:])
            eg2 = wp.tile_from(eg2_d[:])
            eb2 = wp.tile_from(eb2_d[:])
            prew = wp.tile_from(prew_d[:])
            preb = wp.tile_from(preb_d[:])
            embT = wp.tile_from(embT_d[:])
            dw1 = wp.tile_from(dw1_d[:])
            db1 = wp.tile_from(db1_d[:])
            dwa = wp.tile_from(dwa_d[:])
            dwb = wp.tile_from(dwb_d[:])
            dg1 = wp.tile_from(dg1_d[:])
            db1r = wp.tile_from(db1r_d[:])
            dg2 = wp.tile_from(dg2_d[:])
            db2r = wp.tile_from(db2r_d[:])

            cc_i32 = dr.tile([32, 2], f32, name="cc_i32")
            cc_o32 = dr.tile([32, 2], f32, name="cc_o32")
            cc_i128 = dr.tile([128, 2], f32, name="cc_i128")
            cc_o128 = dr.tile([128, 2], f32, name="cc_o128")
            idx0_scr = dr.tile([8192], u16, name="idx0_scr")

            eps_t = sp.tile([128, 1], f32, tag="eps", bufs=1, name="eps_t")
            nc.gpsimd.memset(eps_t[:], 1e-5)
            coll_i0 = sp.tile([128, 64], u16, tag="ci0", bufs=1, name="coll_i0")
            coll_i2 = sp.tile([128, 64], u16, tag="ci2", bufs=1, name="coll_i2")
            coll_sm = sp.tile([128, 64], f32, tag="csm", bufs=1, name="coll_sm")

            def bn_cc(raw, C, g_ap, b_ap, cci, cco, name):
                """raw: SBUF AP [C, 8192] of raw conv outputs (both images).
                Returns (a, c) coefficient APs [C, 1]."""
                st6 = sp.tile([C, 96], f32, tag=f"st6_{C}", name=f"{name}_st6")
                for i in range(16):
                    nc.vector.bn_stats(st6[:, 6 * i:6 * i + 6],
                                       raw[:, 512 * i:512 * (i + 1)])
                mv = sp.tile([C, 2], f32, tag=f"mv_{C}", name=f"{name}_mv")
                nc.vector.bn_aggr(mv[:], st6[:])
                m2 = sp.tile([C, 1], f32, tag=f"m2_{C}", name=f"{name}_m2")
                nc.vector.tensor_tensor(m2[:], mv[:, 0:1], mv[:, 0:1],
                                        op=mybir.AluOpType.mult)
                ex2 = sp.tile([C, 1], f32, tag=f"ex2_{C}", name=f"{name}_ex2")
                nc.vector.tensor_tensor(ex2[:], mv[:, 1:2], m2[:],
                                        op=mybir.AluOpType.add)
                csb = sp.tile([C, 2], f32, tag=f"csb_{C}", name=f"{name}_csb")
                nc.vector.tensor_scalar_mul(csb[:, 0:1], mv[:, 0:1], 8192.0)
                nc.vector.tensor_scalar_mul(csb[:, 1:2], ex2[:], 8192.0)
                nc.sync.dma_start(cci[:], csb[:])
                if with_cc:
                    nc.gpsimd.collective_compute(
                        "AllReduce", mybir.AluOpType.add,
                        replica_groups=[list(range(NCORES))],
                        ins=[cci.opt()], outs=[cco.opt()])
                else:
                    nc.sync.dma_start(cco[:], cci[:])
                gsb = sp.tile([C, 2], f32, tag=f"gsb_{C}", name=f"{name}_gsb")
                nc.sync.dma_start(gsb[:], cco[:])
                gm = sp.tile([C, 1], f32, tag=f"gm_{C}", name=f"{name}_gm")
                nc.vector.tensor_scalar_mul(gm[:], gsb[:, 0:1], 1.0 / 65536.0)
                gex2 = sp.tile([C, 1], f32, tag=f"gex2_{C}", name=f"{name}_gex2")
                nc.vector.tensor_scalar_mul(gex2[:], gsb[:, 1:2], 1.0 / 65536.0)
                gm2 = sp.tile([C, 1], f32, tag=f"gm2_{C}", name=f"{name}_gm2")
                nc.vector.tensor_tensor(gm2[:], gm[:], gm[:],
                                        op=mybir.AluOpType.mult)
                gvar = sp.tile([C, 1], f32, tag=f"gvar_{C}", name=f"{name}_gvar")
                nc.vector.tensor_tensor(gvar[:], gex2[:], gm2[:],
                                        op=mybir.AluOpType.subtract)
                sd = sp.tile([C, 1], f32, tag=f"sd_{C}", name=f"{name}_sd")
                nc.scalar.activation(sd[:], gvar[:], A.Sqrt, bias=eps_t[0:C, :])
                inv = sp.tile([C, 1], f32, tag=f"inv_{C}", name=f"{name}_inv")
                nc.vector.reciprocal(inv[:], sd[:])
                a_t = sp.tile([C, 1], f32, tag=f"a_{C}", name=f"{name}_a")
                nc.vector.tensor_tensor(a_t[:], g_ap, inv[:],
                                        op=mybir.AluOpType.mult)
                am = sp.tile([C, 1], f32, tag=f"am_{C}", name=f"{name}_am")
                nc.vector.tensor_tensor(am[:], a_t[:], gm[:],
                                        op=mybir.AluOpType.mult)
                c_t = sp.tile([C, 1], f32, tag=f"c_{C}", name=f"{name}_c")
                nc.vector.tensor_tensor(c_t[:], b_ap, am[:],
                                        op=mybir.AluOpType.subtract)
                return a_t[:], c_t[:]

            def res_stack(P, pp, xs, wa, wb, g1s, b1s, g2s, b2s, dt, cci32,
                          cco32, cci128, cco128, pre):
                """xs: list of 2 padded-66 x-state tile APs. Returns xs."""
                for blk in range(2):
                    xr = []
                    for i in range(2):
                        t = P.tile([128, 4356], dt, tag="xr", bufs=1,
                                   name=f"{pre}xr{blk}_{i}")
                        for q in range(4):
                            nc.scalar.activation(t[:, 1089 * q:1089 * (q + 1)],
                                                 xs[i][:, 1089 * q:1089 * (q + 1)],
                                                 A.Relu)
                        xr.append(t)
                    rawa = P.tile([32, 8192], f32, tag="rawa", bufs=1,
                                  name=f"{pre}rawa{blk}")

                    for i in range(2):
                        def emit_a(bank, ps, i=i):
                            off = 4096 * i + 512 * bank
                            nc.scalar.activation(rawa[:, off:off + 512], ps[:],
                                                 A.Copy)
                        _conv3x3(nc, pp, xr[i][:],
                                 lambda p: wa[:, (9 * blk + p) * 32:
                                              (9 * blk + p) * 32 + 32],
                                 128, 32, dt, emit_a, f"{pre}ca{blk}_{i}")
                    a1, c1 = bn_cc(rawa[:], 32, g1s[:, blk:blk + 1],
                                   b1s[:, blk:blk + 1], cci32, cco32,
                                   f"{pre}bn1_{blk}")
                    for q in range(16):
                        sl = slice(512 * q, 512 * (q + 1))
                        nc.scalar.activation(rawa[:, sl], rawa[:, sl], A.Relu,
                                             bias=c1, scale=a1)
                    rawb = P.tile([128, 8192], f32, tag="rawb", bufs=1,
                                  name=f"{pre}rawb{blk}")
                    for i in range(2):
                        for bank in range(8):
                            off = 4096 * i + 512 * bank
                            ps = pp.tile([128, 512], f32, tag="ps",
                                         name=f"{pre}cb{blk}_{i}_{bank}")
                            _mm(nc, ps[:],
                                wb[:, 128 * blk:128 * blk + 128],
                                rawa[0:32, off:off + 512], f32, True, True)
                            nc.scalar.activation(rawb[:, off:off + 512],
                                                 ps[:], A.Copy)
                    a2, c2 = bn_cc(rawb[:], 128, g2s[:, blk:blk + 1],
                                   b2s[:, blk:blk + 1], cci128, cco128,
                                   f"{pre}bn2_{blk}")
                    for q in range(16):
                        sl = slice(512 * q, 512 * (q + 1))
                        nc.scalar.activation(rawb[:, sl], rawb[:, sl],
                                             A.Identity, bias=c2, scale=a2)
                    for i in range(2):
                        xv = xs[i][:].rearrange("p (h w) -> p h w", w=66)
                        xint = xv[:, 1:65, 1:65]
                        rv = rawb[:, 4096 * i:4096 * (i + 1)].rearrange(
                            "p (h w) -> p h w", w=64)
                        nc.vector.tensor_tensor(xint, xint, rv,
                                                op=mybir.AluOpType.add)
                return xs

            # ============ P1: conv1 + conv2 + conv3 ============
            xst = []
            for i in range(2):
                t = mp.tile([128, 4356], f32, tag="xst", bufs=2,
                            name=f"xst{i}")
                xst.append(t)
            with tc.tile_pool(name="p1", bufs=1) as p1, \
                 tc.tile_pool(name="pp1", bufs=8, space="PSUM") as pp1:
                for i in range(2):
                    X1 = p1.tile([128, 16900], f32, tag="X1", bufs=1,
                                 name=f"X1_{i}")
                    _memset_rings130(nc, X1[:])
                    X1v = X1[:].rearrange("p (h w) -> p h w", w=130)
                    for c in range(32):
                        stg = p1.tile([16, 512], f32, tag="c1s", bufs=3,
                                      name=f"c1s_{i}_{c}")
                        nc.sync.dma_start(
                            stg[:], xcol_d[:, 16384 * i + 512 * c:
                                           16384 * i + 512 * (c + 1)])
                        ps = pp1.tile([128, 512], f32, tag="ps",
                                      name=f"c1ps_{i}_{c}")
                        _mm(nc, ps[:], w1[:], stg[:], ENC_DT, True, True)
                        psv = ps[:].rearrange("p (a b) -> p a b", b=128)
                        nc.scalar.activation(
                            X1v[0:64, 1 + 4 * c:5 + 4 * c, 1:129],
                            psv[0:64], A.Relu, bias=b1[0:64, :])
                        nc.scalar.activation(
                            X1v[64:128, 1 + 4 * c:5 + 4 * c, 0:128],
                            psv[64:128], A.Relu, bias=b1[64:128, :])
                    c2o = p1.tile([128, 4356], f32, tag="c2o", bufs=1,
                                  name=f"c2o_{i}")
                    _memset_rings66(nc, c2o[:])
                    c2ov = c2o[:].rearrange("p (h w) -> p h w", w=66)
                    for bank in range(8):
                        y0 = 8 * bank
                        ps = pp1.tile([128, 512], f32, tag="ps",
                                      name=f"c2ps_{i}_{bank}")
                        for r in range(8):
                            ky, kxp = r // 2, r % 2
                            rhs = X1v[:, 2 * y0 + ky:2 * y0 + ky + 15:2,
                                      2 * kxp:2 * kxp + 127:2]
                            _mm(nc, ps[:], w2p[:, 128 * r:128 * (r + 1)],
                                rhs, ENC_DT, r == 0, r == 7)
                        psv = ps[:].rearrange("p (a b) -> p a b", b=64)
                        nc.scalar.activation(c2ov[:, 1 + y0:9 + y0, 1:65],
                                             psv, A.Relu, bias=b2[:])
                    _memset_rings66(nc, xst[i][:])
                    xiv = xst[i][:].rearrange("p (h w) -> p h w", w=66)

                    def emit3(bank, ps, xiv=xiv):
                        y0 = 8 * bank
                        psv = ps[:].rearrange("p (a b) -> p a b", b=64)
                        nc.scalar.activation(xiv[:, 1 + y0:9 + y0, 1:65],
                                             psv, A.Identity, bias=b3[:])
                    _conv3x3(nc, pp1, c2o[:],
                             lambda p: w3[:, 128 * p:128 * (p + 1)],
                             128, 128, ENC_DT, emit3, f"c3_{i}")

            # ============ P2a: encoder res stack ============
            with tc.tile_pool(name="p2a", bufs=1) as p2a, \
                 tc.tile_pool(name="pp2a", bufs=8, space="PSUM") as pp2a:
                res_stack(p2a, pp2a, xst, ewa, ewb, eg1, eb1, eg2, eb2,
                          ENC_DT, cc_i32, cc_o32, cc_i128, cc_o128, "e")
                xrf = []
                for i in range(2):
                    t = mp.tile([128, 4356], f32, tag="xrf", bufs=2,
                                name=f"exrf{i}")
                    for q in range(4):
                        nc.scalar.activation(t[:, 1089 * q:1089 * (q + 1)],
                                             xst[i][:, 1089 * q:1089 * (q + 1)],
                                             A.Relu)
                    xrf.append(t)

            # ============ P2b: pre-VQ conv + VQ ============
            with tc.tile_pool(name="p2b", bufs=1) as p2b, \
                 tc.tile_pool(name="pp2b", bufs=2, space="PSUM") as pp2b:
                z = p2b.tile([128, 8192], f32, tag="z", bufs=1, name="z")
                for i in range(2):
                    xv = xrf[i][:].rearrange("p (h w) -> p h w", w=66)
                    for bank in range(8):
                        y0 = 8 * bank
                        ps = pp2b.tile([128, 512], f32, tag="pvps", bufs=2,
                                       name=f"pv_{i}_{bank}")
                        _mm(nc, ps[:], prew[:], xv[:, 1 + y0:9 + y0, 1:65],
                            ENC_DT, True, True)
                        off = 4096 * i + 512 * bank
                        nc.scalar.activation(z[:, off:off + 512], ps[:],
                                             A.Identity, bias=preb[:])
                q0T = p2b.tile([128, 8192], f32, tag="q0T", bufs=1, name="q0T")
                zsq = sp.tile([128, 1], f32, tag="zsq", bufs=1, name="zsq")
                nc.scalar.activation(q0T[:], z[:], A.Square, accum_out=zsq[:])
                nc.sync.dma_start(ozsq_d[:], zsq[:])

                e2neg_t = p2b.tile_from(e2neg_d[:])
                e2rep = p2b.tile([128, 1024], f32, tag="e2rep", bufs=1,
                                 name="e2rep")
                nc.gpsimd.partition_broadcast(e2rep[:], e2neg_t[:])

                for c in range(64):
                    ps = pp2b.tile([128, 1024], f32, tag="vqps", bufs=3,
                                   name=f"vqps{c}")
                    for h in range(2):
                        _mm(nc, ps[:, 512 * h:512 * (h + 1)],
                            z[:, 128 * c:128 * (c + 1)],
                            embT[:, 512 * h:512 * (h + 1)], ENC_DT,
                            True, True)
                    s_t = p2b.tile([128, 1024], f32, tag="s", bufs=4,
                                   name=f"s{c}")
                    nc.scalar.activation(s_t[:, 0:512], ps[:, 0:512], A.Copy)
                    nc.scalar.activation(s_t[:, 512:1024], ps[:, 512:1024],
                                         A.Copy)
                    nc.gpsimd.tensor_tensor(s_t[:], s_t[:], e2rep[:],
                                            op=mybir.AluOpType.add)
                    mx = p2b.tile([128, 8], f32, tag="mx", bufs=3,
                                  name=f"mx{c}")
                    ix = p2b.tile([128, 8], u16, tag="ix", bufs=3,
                                  name=f"ix{c}")
                    nc.vector.max(mx[:], s_t[:])
                    nc.vector.max_index(ix[:], mx[:], s_t[:])
                    nc.gpsimd.tensor_copy(coll_i0[:, c:c + 1], ix[:, 0:1])
                    nc.gpsimd.tensor_copy(coll_i2[:, c:c + 1], ix[:, 2:3])
                    if c % 8 == 7:
                        g0 = c - 7
                        scr_v = idx0_scr[:].rearrange("(p c) -> p c", c=64)
                        nc.sync.dma_start(scr_v[:, g0:g0 + 8],
                                          coll_i0[:, g0:g0 + 8])
                    nc.scalar.activation(coll_sm[:, c:c + 1], mx[:, 0:1],
                                         A.Copy)

                nc.sync.dma_start(oidx2_d[:], coll_i2[:])
                nc.sync.dma_start(osmax_d[:], coll_sm[:])
                wrapped = p2b.tile([128, 512], u16, tag="wrapped", bufs=1,
                                   name="wrapped")
                # slots 64*vg..64*vg+64 <=> chunks 8*vg..8*vg+8 (vecs 1024*vg..)
                srcv = idx0_scr[:].rearrange("(q r c) -> r c q", q=8, r=16,
                                             c=64)
                for g in range(8):
                    dst = wrapped[16 * g:16 * (g + 1), :].rearrange(
                        "r (c q) -> r c q", q=8)
                    nc.sync.dma_start(dst, srcv)
                nc.gpsimd.ap_gather(q0T[:], embT[:],
                                    wrapped[:].bitcast(i16), channels=128,
                                    num_elems=1024, d=1, num_idxs=8192)

                # quantized -> decoder input (padded, reuse xrf slots)
                q0pad = []
                for i in range(2):
                    t = mp.tile([128, 4356], DEC_DT, tag="xrf", bufs=2,
                                name=f"q0pad{i}")
                    _memset_rings66(nc, t[:])
                    tv = t[:].rearrange("p (h w) -> p h w", w=66)
                    qv = q0T[:, 4096 * i:4096 * (i + 1)].rearrange(
                        "p (h w) -> p h w", w=64)
                    for q in range(4):
                        nc.scalar.activation(
                            tv[:, 1 + 16 * q:1 + 16 * (q + 1), 1:65],
                            qv[:, 16 * q:16 * (q + 1), :], A.Copy)
                    q0pad.append(t)

            # ============ P3a: d_w1 conv + decoder res stack ============
            with tc.tile_pool(name="p3a", bufs=1) as p3a, \
                 tc.tile_pool(name="pp3a", bufs=8, space="PSUM") as pp3a:
                if DEC_DT != f32:
                    dw1r = p3a.tile([128, 9 * 128], DEC_DT, tag="dw1r",
                                    bufs=1, name="dw1r")
                    nc.scalar.activation(dw1r[:], dw1[:], A.Copy)
                    dwar = p3a.tile([128, 2 * 9 * 32], DEC_DT, tag="dwar",
                                    bufs=1, name="dwar")
                    nc.scalar.activation(dwar[:], dwa[:], A.Copy)
                else:
                    dw1r, dwar = dw1, dwa
                for i in range(2):
                    _memset_rings66(nc, xst[i][:])
                    yv = xst[i][:].rearrange("p (h w) -> p h w", w=66)

                    def emitd(bank, ps, yv=yv):
                        y0 = 8 * bank
                        psv = ps[:].rearrange("p (a b) -> p a b", b=64)
                        nc.scalar.activation(yv[:, 1 + y0:9 + y0, 1:65],
                                             psv, A.Identity, bias=db1[:])
                    _conv3x3(nc, pp3a, q0pad[i][:],
                             lambda p: dw1r[:, 128 * p:128 * (p + 1)],
                             128, 128, DEC_DT, emitd, f"dw1_{i}")
                res_stack(p3a, pp3a, xst, dwar, dwb, dg1, db1r, dg2, db2r,
                          DEC_DT, cc_i32, cc_o32, cc_i128, cc_o128, "d")
                yrf = []
                for i in range(2):
                    t = mp.tile([128, 4356], DEC_DT, tag="xrf", bufs=2,
                                name=f"dxrf{i}")
                    for q in range(4):
                        nc.scalar.activation(t[:, 1089 * q:1089 * (q + 1)],
                                             xst[i][:, 1089 * q:1089 * (q + 1)],
                                             A.Relu)
                    yrf.append(t)

            # ============ P3b: dt1 + dt2 ============
            with tc.tile_pool(name="p3b", bufs=1) as p3b, \
                 tc.tile_pool(name="pp3b", bufs=8, space="PSUM") as pp3b:
                dt1w = p3b.tile_from(dt1w_d[:])
                dt1b = p3b.tile_from(dt1b_d[:])
                dt2w = p3b.tile_from(dt2w_d[:])
                if DEC_DT != f32:
                    dt1wr = p3b.tile([128, 16 * 128], DEC_DT, tag="dt1wr",
                                     bufs=1, name="dt1wr")
                    nc.scalar.activation(dt1wr[:], dt1w[:], A.Copy)
                    dt2wr = p3b.tile([128, 24], DEC_DT, tag="dt2wr",
                                     bufs=1, name="dt2wr")
                    nc.scalar.activation(dt2wr[:], dt2w[:], A.Copy)
                else:
                    dt1wr, dt2wr = dt1w, dt2w
                for i in range(2):
                    X2 = p3b.tile([128, 16900], DEC_DT, tag="X2", bufs=1,
                                  name=f"X2_{i}")
                    _memset_rings130(nc, X2[:])
                    X2v = X2[:].rearrange("p (h w) -> p h w", w=130)
                    yv = yrf[i][:].rearrange("p (h w) -> p h w", w=66)
                    for a in range(2):
                        for b in range(2):
                            for bank in range(8):
                                u0 = 8 * bank
                                ps = pp3b.tile([128, 512], f32, tag="ps",
                                               name=f"t1_{i}_{a}{b}_{bank}")
                                k = 0
                                for t_i, (ky, sy) in enumerate(TAPS[a]):
                                    for s_i, (kx, sx) in enumerate(TAPS[b]):
                                        idx = ((a * 2 + b) * 2 + t_i) * 2 + s_i
                                        rhs = yv[:, 1 + u0 - sy:9 + u0 - sy,
                                                 1 - sx:65 - sx]
                                        _mm(nc, ps[:],
                                            dt1wr[:, 128 * idx:128 * (idx + 1)],
                                            rhs, DEC_DT, k == 0, k == 3)
                                        k += 1
                                psv = ps[:].rearrange("p (a b) -> p a b", b=64)
                                r0 = 1 + 2 * u0 + a
                                nc.scalar.activation(
                                    X2v[0:64, r0:r0 + 16:2,
                                        1 + b:129 + b:2],
                                    psv[0:64], A.Relu, bias=dt1b[0:64, :])
                                nc.scalar.activation(
                                    X2v[64:128, r0:r0 + 16:2, b:128 + b:2],
                                    psv[64:128], A.Relu, bias=dt1b[64:128, :])
                    for ch in range(32):
                        u0 = 4 * ch
                        ps2 = pp3b.tile([4, 512], f32, tag="ps",
                                        name=f"t2_{i}_{ch}")
                        for syi, sy in enumerate((-1, 0, 1)):
                            r0 = 1 + u0 - sy
                            rhs = X2v[0:128, r0:r0 + 4, 1:129]
                            _mm(nc, ps2[:], dt2wr[:, 8 * syi:8 * syi + 4],
                                rhs, DEC_DT, syi == 0, False)
                            rhs2 = X2v[0:64, r0:r0 + 4, 0:128]
                            _mm(nc, ps2[:],
                                dt2wr[0:64, 8 * syi + 4:8 * syi + 8],
                                rhs2, DEC_DT, False, syi == 2)
                        yst = p3b.tile([4, 512], f32, tag="yst", bufs=3,
                                       name=f"yst_{i}_{ch}")
                        nc.vector.tensor_copy(yst[:], ps2[:])
                        nc.sync.dma_start(
                            outy_d[i:i + 1, :, 512 * ch:512 * (ch + 1)],
                            yst[:])
    nc.compile()
    return nc


def _prep_inputs(inp):
    F = np.float32
    g = {k: np.ascontiguousarray(np.asarray(v), dtype=F) for k, v in inp.items()
         if k != 'emb'}
    g['emb'] = np.ascontiguousarray(np.asarray(inp['emb']), dtype=F)
    x = g['x']

    shared = {}
    shared['w1'] = np.concatenate([g['p_w1'].reshape(64, 16).T] * 2, axis=1)
    shared['b1'] = np.concatenate([g['p_b1']] * 2)[:, None]

    w2 = g['p_w2']  # [128, 64, 4, 4]
    w2p = np.zeros((128, 8, 128), F)
    for r in range(8):
        ky, kxp = r // 2, r % 2
        w2p[0:64, r] = w2[:, :, ky, 2 * kxp].T
        w2p[64:128, r] = w2[:, :, ky, 2 * kxp + 1].T
    shared['w2p'] = w2p.reshape(128, 1024)
    shared['b2'] = g['p_b2'][:, None]

    def conv9(w):  # [Cout, Cin, 3, 3] -> [Cin, 9*Cout]
        Cout, Cin = w.shape[0], w.shape[1]
        out = np.zeros((Cin, 9, Cout), F)
        for p, (ky, kx) in enumerate(POS9):
            out[:, p] = w[:, :, ky, kx].T
        return out.reshape(Cin, 9 * Cout)

    shared['w3'] = conv9(g['p_w3'])
    shared['b3'] = g['p_b3'][:, None]

    def resw(wa, wb):
        ewa = np.zeros((128, 2, 9, 32), F)
        for blk in range(2):
            ewa[:, blk] = conv9(wa[blk]).reshape(128, 9, 32)
        ewb = np.zeros((32, 2, 128), F)
        for blk in range(2):
            ewb[:, blk] = wb[blk][:, :, 0, 0].T
        return ewa.reshape(128, 576), ewb.reshape(32, 256)

    shared['ewa'], shared['ewb'] = resw(g['pr_wa'], g['pr_wb'])
    shared['eg1'] = g['pr_g1'].T.copy()
    shared['eb1'] = g['pr_b1'].T.copy()
    shared['eg2'] = g['pr_g2'].T.copy()
    shared['eb2'] = g['pr_b2'].T.copy()
    shared['prew'] = g['pre_w'][:, :, 0, 0].T.copy()
    shared['preb'] = g['pre_b'][:, None]
    emb = g['emb']
    shared['embT'] = emb.T.copy()
    shared['e2neg'] = (-0.5 * (emb.astype(F) ** 2).sum(1))[None, :].astype(F)
    shared['dw1'] = conv9(g['d_w1'])
    shared['db1'] = g['d_b1'][:, None]
    shared['dwa'], shared['dwb'] = resw(g['dr_wa'], g['dr_wb'])
    shared['dg1'] = g['dr_g1'].T.copy()
    shared['db1r'] = g['dr_b1'].T.copy()
    shared['dg2'] = g['dr_g2'].T.copy()
    shared['db2r'] = g['dr_b2'].T.copy()

    wt1 = g['dt1_w']  # [128, 64, 4, 4] (Cin, Cout, kh, kw)
    dt1w = np.zeros((128, 16, 128), F)
    for a in range(2):
        for b in range(2):
            for t_i, (ky, sy) in enumerate(TAPS[a]):
                for s_i, (kx, sx) in enumerate(TAPS[b]):
                    idx = ((a * 2 + b) * 2 + t_i) * 2 + s_i
                    dt1w[:, idx, 0:64] = wt1[:, :, ky, kx]
                    dt1w[:, idx, 64:128] = wt1[:, :, ky, kx]
    shared['dt1w'] = dt1w.reshape(128, 2048)
    shared['dt1b'] = np.concatenate([g['dt1_b']] * 2)[:, None]

    wt2 = g['dt2_w']  # [64, 1, 4, 4]
    ky_of = {0: {0: 1, 1: 3}, 1: {-1: 0, 0: 2}}  # [a][sy] -> ky
    kx_of = {0: {0: 1, 1: 3}, 1: {-1: 0, 0: 2}}
    dt2w = np.zeros((128, 6, 4), F)
    for syi, sy in enumerate((-1, 0, 1)):
        for a in range(2):
            if sy not in ky_of[a]:
                continue
            ky = ky_of[a][sy]
            for b in range(2):
                m = 2 * a + b
                # pair round: rows 0-63 sx=0, rows 64-127 sx=-1
                if 0 in kx_of[b]:
                    dt2w[0:64, 2 * syi, m] = wt2[:, 0, ky, kx_of[b][0]]
                if -1 in kx_of[b]:
                    dt2w[64:128, 2 * syi, m] = wt2[:, 0, ky, kx_of[b][-1]]
                # solo round: rows 0-63 sx=+1
                if 1 in kx_of[b]:
                    dt2w[0:64, 2 * syi + 1, m] = wt2[:, 0, ky, kx_of[b][1]]
    shared['dt2w'] = dt2w.reshape(128, 24)

    in_maps = []
    for core in range(NCORES):
        m = dict(shared)
        cols = np.zeros((16, 2, 128, 128), F)
        for i in range(2):
            img = x[2 * core + i, 0]
            xp = np.zeros((258, 258), F)
            xp[1:257, 1:257] = img
            for ky in range(4):
                for kx in range(4):
                    cols[ky * 4 + kx, i] = xp[ky:ky + 256:2, kx:kx + 256:2][:128, :128]
        m['xcol'] = cols.reshape(16, 2 * 16384)
        in_maps.append(m)
    return in_maps, g


_NC_CACHE = {}


def kernel(**inputs):
    in_maps, g = _prep_inputs(inputs)
    if 'nc' not in _NC_CACHE:
        _NC_CACHE['nc'] = _build()
    nc = _NC_CACHE['nc']
    res = run_bass_kernel_spmd(nc, in_maps, list(range(NCORES)))
    results = res.results

    F = np.float32
    x_recon = np.zeros((16, 1, 256, 256), F)
    dt2b = float(g['dt2_b'][0])
    tot = 0.0
    idx2_all = []
    for core in range(NCORES):
        r = results[core]
        y = r['out_y'].reshape(2, 4, 128, 128)
        for i in range(2):
            for a in range(2):
                for b in range(2):
                    x_recon[2 * core + i, 0, a::2, b::2] = y[i, 2 * a + b]
        tot += float(r['out_zsq'].astype(np.float64).sum())
        tot -= 2.0 * float(r['out_smax'].astype(np.float64).sum())
        idx2_all.append(r['out_idx2'].T.reshape(-1))
    x_recon += F(dt2b)

    e_latent = tot / (16 * 4096 * 128)
    loss = F(0.25 * e_latent)

    counts = np.bincount(np.concatenate(idx2_all).astype(np.int64),
                         minlength=1024).astype(np.float64)
    avg = counts / (16 * 4096)
    perp = F(np.exp(-np.sum(avg * np.log(avg + 1e-10))))
    return loss, x_recon, perp


# revision 29
# speedup vs baseline: 1.0280x; 1.0280x over previous
"""BeamVQ (VQ-VAE fwd) Trainium2 kernel — 8-core batch-parallel.

Strategy: shard batch 16 -> 8 cores x 2 images. Convs as shift-matmuls on PE
(channels on partitions, PSUM accumulation). Training-mode BN via per-channel
sum/sumsq with a tiny in-kernel AllReduce per BN layer (8 total). VQ: PE fp32
distance scores s = z.e - |e|^2/2, DVE max/max_index top-8, gpsimd ap_gather
codebook lookup. ConvTranspose via output-parity decomposition. Loss /
perplexity partial sums are finished on the host.

Precision: encoder + pre-VQ + distance matmuls in true fp32 (top-1 gaps go
down to 5e-5; fp32r's ~1e-5 operand rounding flips argmins and corrupts the
reconstruction by O(1)). Decoder matmuls run in fp32r (4x faster PE streams;
weights/activations are rounded to f32r by their ACT producers as the BIR
verifier requires) giving recon absmax ~1.4e-3 on scale 3.7; set DEC_DT = f32
to get absmax ~6e-6 at +~550us.

Measured (8 trn2 cores via axon/PJRT): recon absmax 1.38e-3 (rel 3.8e-4),
loss rel 3.5e-7, perplexity rel 5.5e-6 vs the fp32 CPU reference.
Cost-model timeline estimate ~1.36 ms end-to-end (excl. AllReduce latency).
"""
import numpy as np

import concourse.bass as bass
import concourse.mybir as mybir
from concourse import bacc, tile
from concourse.bass_utils import run_bass_kernel_spmd

f32 = mybir.dt.float32
f32r = mybir.dt.float32r
u16 = mybir.dt.uint16
i16 = mybir.dt.int16

NCORES = 8
ENC_DT = f32    # z-path matmul dtype (must stay fp32)
DEC_DT = f32r   # decoder matmul dtype (operands produced as rounded f32r)

POS9 = [(ky, kx) for ky in range(3) for kx in range(3)]
# transposed-conv k4 s2 p1 parity taps: parity -> [(k, shift)]
TAPS = {0: [(1, 0), (3, 1)], 1: [(0, -1), (2, 0)]}

A = mybir.ActivationFunctionType


def _mm(nc, ps, lhsT, rhs, dt, start, stop):
    # operands must already carry dtype `dt` (f32r data must be produced
    # rounded — the BIR verifier rejects plain bitcasts of f32 data)
    nc.tensor.matmul(ps, lhsT, rhs, start=start, stop=stop)


def _conv3x3(nc, pp, src, wsl, Cin, Cout, dt, emit, name):
    """src: padded-66 AP [>=Cin, 4356]; wsl(pos) -> lhsT [Cin, Cout];
    emit(bank, ps) consumes psum [Cout, 512] for out rows 8*bank..8*bank+8."""
    v = src.rearrange("p (h w) -> p h w", w=66)
    for bank in range(8):
        y0 = 8 * bank
        ps = pp.tile([Cout, 512], f32, tag="ps", name=f"{name}_ps{bank}")
        for pos, (ky, kx) in enumerate(POS9):
            rhs = v[0:Cin, y0 + ky:y0 + ky + 8, kx:kx + 64]
            _mm(nc, ps[:], wsl(pos), rhs, dt, pos == 0, pos == 8)
        emit(bank, ps)


def _memset_rings66(nc, t):
    v = t.bitcast(mybir.dt.uint32).rearrange("p (h w) -> p h w", w=66)
    nc.gpsimd.memset(v[:, 0:66:65, :], 0)
    nc.gpsimd.memset(v[:, :, 0:1], 0)
    nc.gpsimd.memset(v[:, :, 65:66], 0)


def _memset_rings130(nc, t):
    v = t.bitcast(mybir.dt.uint32).rearrange("p (h w) -> p h w", w=130)
    nc.gpsimd.memset(v[:, 0:130:129, :], 0)
    nc.gpsimd.memset(v[:, :, 0:1], 0)
    nc.gpsimd.memset(v[:, :, 128:130], 0)


def _build(with_cc=True):
    nc = bacc.Bacc("TRN2", target_bir_lowering=False, debug=False,
                   enable_asserts=False,
                   num_devices=NCORES if with_cc else 1)

    def din(name, shape, dt=f32):
        return nc.dram_tensor(name, shape, dt, kind="ExternalInput")

    def dout(name, shape, dt=f32):
        return nc.dram_tensor(name, shape, dt, kind="ExternalOutput")

    xcol_d = din("xcol", [16, 2 * 16384])
    w1_d = din("w1", [16, 128])
    b1_d = din("b1", [128, 1])
    w2p_d = din("w2p", [128, 8 * 128])
    b2_d = din("b2", [128, 1])
    w3_d = din("w3", [128, 9 * 128])
    b3_d = din("b3", [128, 1])
    ewa_d = din("ewa", [128, 2 * 9 * 32])
    ewb_d = din("ewb", [32, 2 * 128])
    eg1_d = din("eg1", [32, 2])
    eb1_d = din("eb1", [32, 2])
    eg2_d = din("eg2", [128, 2])
    eb2_d = din("eb2", [128, 2])
    prew_d = din("prew", [128, 128])
    preb_d = din("preb", [128, 1])
    embT_d = din("embT", [128, 1024])
    e2neg_d = din("e2neg", [1, 1024])
    dw1_d = din("dw1", [128, 9 * 128])
    db1_d = din("db1", [128, 1])
    dwa_d = din("dwa", [128, 2 * 9 * 32])
    dwb_d = din("dwb", [32, 2 * 128])
    dg1_d = din("dg1", [32, 2])
    db1r_d = din("db1r", [32, 2])
    dg2_d = din("dg2", [128, 2])
    db2r_d = din("db2r", [128, 2])
    dt1w_d = din("dt1w", [128, 16 * 128])
    dt1b_d = din("dt1b", [128, 1])
    dt2w_d = din("dt2w", [128, 24])

    outy_d = dout("out_y", [2, 4, 16384])
    osmax_d = dout("out_smax", [128, 64])
    oidx2_d = dout("out_idx2", [128, 64], u16)
    ozsq_d = dout("out_zsq", [128, 1])

    with tile.TileContext(nc) as tc:
        with tc.tile_pool(name="wpool", bufs=1) as wp, \
             tc.tile_pool(name="main", bufs=1) as mp, \
             tc.tile_pool(name="small", bufs=2) as sp, \
             tc.tile_pool(name="dram", bufs=1, space="DRAM") as dr:

            w1 = wp.tile_from(w1_d[:])
            b1 = wp.tile_from(b1_d[:])
            w2p = wp.tile_from(w2p_d[:])
            b2 = wp.tile_from(b2_d[:])
            w3 = wp.tile_from(w3_d[:])
            b3 = wp.tile_from(b3_d[:])
            ewa = wp.tile_from(ewa_d[:])
            ewb = wp.tile_from(ewb_d[:])
            eg1 = wp.tile_from(eg1_d[:])
            eb1 = wp.tile_from(eb1_d[:])
            eg2 = wp.tile_from(eg2_d[:])
            eb2 = wp.tile_from(eb2_d[:])
            prew = wp.tile_from(prew_d[:])
            preb = wp.tile_from(preb_d[:])
            embT = wp.tile_from(embT_d[:])
            dw1 = wp.tile_from(dw1_d[:])
            db1 = wp.tile_from(db1_d[:])
            dwa = wp.tile_from(dwa_d[:])
            dwb = wp.tile_from(dwb_d[:])
            dg1 = wp.tile_from(dg1_d[:])
            db1r = wp.tile_from(db1r_d[:])
            dg2 = wp.tile_from(dg2_d[:])
            db2r = wp.tile_from(db2r_d[:])

            cc_i32 = dr.tile([32, 2], f32, name="cc_i32")
            cc_o32 = dr.tile([32, 2], f32, name="cc_o32")
            cc_i128 = dr.tile([128, 2], f32, name="cc_i128")
            cc_o128 = dr.tile([128, 2], f32, name="cc_o128")
            idx0_scr = dr.tile([8192], u16, name="idx0_scr")

            eps_t = sp.tile([128, 1], f32, tag="eps", bufs=1, name="eps_t")
            nc.gpsimd.memset(eps_t[:], 1e-5)
            coll_i0 = sp.tile([128, 64], u16, tag="ci0", bufs=1, name="coll_i0")
            coll_i2 = sp.tile([128, 64], u16, tag="ci2", bufs=1, name="coll_i2")
            coll_sm = sp.tile([128, 64], f32, tag="csm", bufs=1, name="coll_sm")

            def bn_cc(raw, C, g_ap, b_ap, cci, cco, name, pp=None):
                """raw: SBUF AP [C, 8192] of raw conv outputs (both images).
                Returns (a, c) coefficient APs [C, 1]."""
                st6 = sp.tile([C, 96], f32, tag=f"st6_{C}", name=f"{name}_st6")
                for i in range(16):
                    nc.vector.bn_stats(st6[:, 6 * i:6 * i + 6],
                                       raw[:, 512 * i:512 * (i + 1)])
                mv = sp.tile([C, 2], f32, tag=f"mv_{C}", name=f"{name}_mv")
                nc.vector.bn_aggr(mv[:], st6[:])
                m2 = sp.tile([C, 1], f32, tag=f"m2_{C}", name=f"{name}_m2")
                nc.vector.tensor_tensor(m2[:], mv[:, 0:1], mv[:, 0:1],
                                        op=mybir.AluOpType.mult)
                ex2 = sp.tile([C, 1], f32, tag=f"ex2_{C}", name=f"{name}_ex2")
                nc.vector.tensor_tensor(ex2[:], mv[:, 1:2], m2[:],
                                        op=mybir.AluOpType.add)
                csb = sp.tile([C, 2], f32, tag=f"csb_{C}", name=f"{name}_csb")
                nc.vector.tensor_scalar_mul(csb[:, 0:1], mv[:, 0:1], 8192.0)
                nc.vector.tensor_scalar_mul(csb[:, 1:2], ex2[:], 8192.0)
                nc.sync.dma_start(cci[:], csb[:])
                if with_cc:
                    nc.gpsimd.collective_compute(
                        "AllReduce", mybir.AluOpType.add,
                        replica_groups=[list(range(NCORES))],
                        ins=[cci.opt()], outs=[cco.opt()])
                else:
                    nc.sync.dma_start(cco[:], cci[:])
                if pp is not None:
                    fill = pp.tile([128, 512], f32, tag="fill", bufs=1,
                                   name=f"{name}_fill")
                    for k in range(12):
                        nc.tensor.matmul(fill[:], embT[:, 0:128],
                                         embT[:, 0:512], start=True,
                                         stop=True)
                gsb = sp.tile([C, 2], f32, tag=f"gsb_{C}", name=f"{name}_gsb")
                nc.sync.dma_start(gsb[:], cco[:])
                gm = sp.tile([C, 1], f32, tag=f"gm_{C}", name=f"{name}_gm")
                nc.vector.tensor_scalar_mul(gm[:], gsb[:, 0:1], 1.0 / 65536.0)
                gex2 = sp.tile([C, 1], f32, tag=f"gex2_{C}", name=f"{name}_gex2")
                nc.vector.tensor_scalar_mul(gex2[:], gsb[:, 1:2], 1.0 / 65536.0)
                gm2 = sp.tile([C, 1], f32, tag=f"gm2_{C}", name=f"{name}_gm2")
                nc.vector.tensor_tensor(gm2[:], gm[:], gm[:],
                                        op=mybir.AluOpType.mult)
                gvar = sp.tile([C, 1], f32, tag=f"gvar_{C}", name=f"{name}_gvar")
                nc.vector.tensor_tensor(gvar[:], gex2[:], gm2[:],
                                        op=mybir.AluOpType.subtract)
                sd = sp.tile([C, 1], f32, tag=f"sd_{C}", name=f"{name}_sd")
                nc.scalar.activation(sd[:], gvar[:], A.Sqrt, bias=eps_t[0:C, :])
                inv = sp.tile([C, 1], f32, tag=f"inv_{C}", name=f"{name}_inv")
                nc.vector.reciprocal(inv[:], sd[:])
                a_t = sp.tile([C, 1], f32, tag=f"a_{C}", name=f"{name}_a")
                nc.vector.tensor_tensor(a_t[:], g_ap, inv[:],
                                        op=mybir.AluOpType.mult)
                am = sp.tile([C, 1], f32, tag=f"am_{C}", name=f"{name}_am")
                nc.vector.tensor_tensor(am[:], a_t[:], gm[:],
                                        op=mybir.AluOpType.mult)
                c_t = sp.tile([C, 1], f32, tag=f"c_{C}", name=f"{name}_c")
                nc.vector.tensor_tensor(c_t[:], b_ap, am[:],
                                        op=mybir.AluOpType.subtract)
                return a_t[:], c_t[:]

            def res_stack(P, pp, xs, wa, wb, g1s, b1s, g2s, b2s, dt, cci32,
                          cco32, cci128, cco128, pre):
                """xs: list of 2 padded-66 x-state tile APs. Returns xs."""
                for blk in range(2):
                    xr = []
                    for i in range(2):
                        t = P.tile([128, 4356], dt, tag="xr", bufs=1,
                                   name=f"{pre}xr{blk}_{i}")
                        for q in range(4):
                            nc.scalar.activation(t[:, 1089 * q:1089 * (q + 1)],
                                                 xs[i][:, 1089 * q:1089 * (q + 1)],
                                                 A.Relu)
                        xr.append(t)
                    rawa = P.tile([32, 8192], f32, tag="rawa", bufs=1,
                                  name=f"{pre}rawa{blk}")

                    for i in range(2):
                        def emit_a(bank, ps, i=i):
                            off = 4096 * i + 512 * bank
                            nc.scalar.activation(rawa[:, off:off + 512], ps[:],
                                                 A.Copy)
                        _conv3x3(nc, pp, xr[i][:],
                                 lambda p: wa[:, (9 * blk + p) * 32:
                                              (9 * blk + p) * 32 + 32],
                                 128, 32, dt, emit_a, f"{pre}ca{blk}_{i}")
                    a1, c1 = bn_cc(rawa[:], 32, g1s[:, blk:blk + 1],
                                   b1s[:, blk:blk + 1], cci32, cco32,
                                   f"{pre}bn1_{blk}", pp=pp)
                    for q in range(16):
                        sl = slice(512 * q, 512 * (q + 1))
                        nc.scalar.activation(rawa[:, sl], rawa[:, sl], A.Relu,
                                             bias=c1, scale=a1)
                    rawb = P.tile([128, 8192], f32, tag="rawb", bufs=1,
                                  name=f"{pre}rawb{blk}")
                    for i in range(2):
                        for bank in range(8):
                            off = 4096 * i + 512 * bank
                            ps = pp.tile([128, 512], f32, tag="ps",
                                         name=f"{pre}cb{blk}_{i}_{bank}")
                            _mm(nc, ps[:],
                                wb[:, 128 * blk:128 * blk + 128],
                                rawa[0:32, off:off + 512], f32, True, True)
                            nc.scalar.activation(rawb[:, off:off + 512],
                                                 ps[:], A.Copy)
                    a2, c2 = bn_cc(rawb[:], 128, g2s[:, blk:blk + 1],
                                   b2s[:, blk:blk + 1], cci128, cco128,
                                   f"{pre}bn2_{blk}", pp=pp)
                    for q in range(16):
                        sl = slice(512 * q, 512 * (q + 1))
                        nc.scalar.activation(rawb[:, sl], rawb[:, sl],
                                             A.Identity, bias=c2, scale=a2)
                    for i in range(2):
                        xv = xs[i][:].rearrange("p (h w) -> p h w", w=66)
                        xint = xv[:, 1:65, 1:65]
                        rv = rawb[:, 4096 * i:4096 * (i + 1)].rearrange(
                            "p (h w) -> p h w", w=64)
                        nc.vector.tensor_tensor(xint, xint, rv,
                                                op=mybir.AluOpType.add)
                return xs

            # ============ P1: conv1 + conv2 + conv3 ============
            xst = []
            for i in range(2):
                t = mp.tile([128, 4356], f32, tag="xst", bufs=2,
                            name=f"xst{i}")
                xst.append(t)
            with tc.tile_pool(name="p1", bufs=1) as p1, \
                 tc.tile_pool(name="pp1", bufs=8, space="PSUM") as pp1:
                for i in range(2):
                    X1 = p1.tile([128, 16900], f32, tag="X1", bufs=1,
                                 name=f"X1_{i}")
                    _memset_rings130(nc, X1[:])
                    X1v = X1[:].rearrange("p (h w) -> p h w", w=130)
                    for c in range(32):
                        stg = p1.tile([16, 512], f32, tag="c1s", bufs=3,
                                      name=f"c1s_{i}_{c}")
                        nc.sync.dma_start(
                            stg[:], xcol_d[:, 16384 * i + 512 * c:
                                           16384 * i + 512 * (c + 1)])
                        ps = pp1.tile([128, 512], f32, tag="ps",
                                      name=f"c1ps_{i}_{c}")
                        _mm(nc, ps[:], w1[:], stg[:], ENC_DT, True, True)
                        psv = ps[:].rearrange("p (a b) -> p a b", b=128)
                        nc.scalar.activation(
                            X1v[0:64, 1 + 4 * c:5 + 4 * c, 1:129],
                            psv[0:64], A.Relu, bias=b1[0:64, :])
                        nc.scalar.activation(
                            X1v[64:128, 1 + 4 * c:5 + 4 * c, 0:128],
                            psv[64:128], A.Relu, bias=b1[64:128, :])
                    c2o = p1.tile([128, 4356], f32, tag="c2o", bufs=1,
                                  name=f"c2o_{i}")
                    _memset_rings66(nc, c2o[:])
                    c2ov = c2o[:].rearrange("p (h w) -> p h w", w=66)
                    for bank in range(8):
                        y0 = 8 * bank
                        ps = pp1.tile([128, 512], f32, tag="ps",
                                      name=f"c2ps_{i}_{bank}")
                        for r in range(8):
                            ky, kxp = r // 2, r % 2
                            rhs = X1v[:, 2 * y0 + ky:2 * y0 + ky + 15:2,
                                      2 * kxp:2 * kxp + 127:2]
                            _mm(nc, ps[:], w2p[:, 128 * r:128 * (r + 1)],
                                rhs, ENC_DT, r == 0, r == 7)
                        psv = ps[:].rearrange("p (a b) -> p a b", b=64)
                        nc.scalar.activation(c2ov[:, 1 + y0:9 + y0, 1:65],
                                             psv, A.Relu, bias=b2[:])
                    _memset_rings66(nc, xst[i][:])
                    xiv = xst[i][:].rearrange("p (h w) -> p h w", w=66)

                    def emit3(bank, ps, xiv=xiv):
                        y0 = 8 * bank
                        psv = ps[:].rearrange("p (a b) -> p a b", b=64)
                        nc.scalar.activation(xiv[:, 1 + y0:9 + y0, 1:65],
                                             psv, A.Identity, bias=b3[:])
                    _conv3x3(nc, pp1, c2o[:],
                             lambda p: w3[:, 128 * p:128 * (p + 1)],
                             128, 128, ENC_DT, emit3, f"c3_{i}")

            # ============ P2a: encoder res stack ============
            with tc.tile_pool(name="p2a", bufs=1) as p2a, \
                 tc.tile_pool(name="pp2a", bufs=7, space="PSUM") as pp2a:
                res_stack(p2a, pp2a, xst, ewa, ewb, eg1, eb1, eg2, eb2,
                          ENC_DT, cc_i32, cc_o32, cc_i128, cc_o128, "e")
                xrf = []
                for i in range(2):
                    t = mp.tile([128, 4356], f32, tag="xrf", bufs=2,
                                name=f"exrf{i}")
                    for q in range(4):
                        nc.scalar.activation(t[:, 1089 * q:1089 * (q + 1)],
                                             xst[i][:, 1089 * q:1089 * (q + 1)],
                                             A.Relu)
                    xrf.append(t)

            # ============ P2b: pre-VQ conv + VQ ============
            with tc.tile_pool(name="p2b", bufs=1) as p2b, \
                 tc.tile_pool(name="pp2b", bufs=2, space="PSUM") as pp2b:
                z = p2b.tile([128, 8192], f32, tag="z", bufs=1, name="z")
                for i in range(2):
                    xv = xrf[i][:].rearrange("p (h w) -> p h w", w=66)
                    for bank in range(8):
                        y0 = 8 * bank
                        ps = pp2b.tile([128, 512], f32, tag="pvps", bufs=2,
                                       name=f"pv_{i}_{bank}")
                        _mm(nc, ps[:], prew[:], xv[:, 1 + y0:9 + y0, 1:65],
                            ENC_DT, True, True)
                        off = 4096 * i + 512 * bank
                        nc.scalar.activation(z[:, off:off + 512], ps[:],
                                             A.Identity, bias=preb[:])
                q0T = p2b.tile([128, 8192], f32, tag="q0T", bufs=1, name="q0T")
                zsq = sp.tile([128, 1], f32, tag="zsq", bufs=1, name="zsq")
                nc.scalar.activation(q0T[:], z[:], A.Square, accum_out=zsq[:])
                nc.sync.dma_start(ozsq_d[:], zsq[:])

                e2neg_t = p2b.tile_from(e2neg_d[:])
                e2rep = p2b.tile([128, 1024], f32, tag="e2rep", bufs=1,
                                 name="e2rep")
                nc.gpsimd.partition_broadcast(e2rep[:], e2neg_t[:])

                for c in range(64):
                    ps = pp2b.tile([128, 1024], f32, tag="vqps", bufs=3,
                                   name=f"vqps{c}")
                    for h in range(2):
                        _mm(nc, ps[:, 512 * h:512 * (h + 1)],
                            z[:, 128 * c:128 * (c + 1)],
                            embT[:, 512 * h:512 * (h + 1)], ENC_DT,
                            True, True)
                    s_t = p2b.tile([128, 1024], f32, tag="s", bufs=4,
                                   name=f"s{c}")
                    nc.scalar.activation(s_t[:, 0:512], ps[:, 0:512], A.Copy)
                    nc.scalar.activation(s_t[:, 512:1024], ps[:, 512:1024],
                                         A.Copy)
                    nc.gpsimd.tensor_tensor(s_t[:], s_t[:], e2rep[:],
                                            op=mybir.AluOpType.add)
                    mx = p2b.tile([128, 8], f32, tag="mx", bufs=3,
                                  name=f"mx{c}")
                    ix = p2b.tile([128, 8], u16, tag="ix", bufs=3,
                                  name=f"ix{c}")
                    nc.vector.max(mx[:], s_t[:])
                    nc.vector.max_index(ix[:], mx[:], s_t[:])
                    nc.gpsimd.tensor_copy(coll_i0[:, c:c + 1], ix[:, 0:1])
                    nc.gpsimd.tensor_copy(coll_i2[:, c:c + 1], ix[:, 2:3])
                    if c % 8 == 7:
                        g0 = c - 7
                        scr_v = idx0_scr[:].rearrange("(p c) -> p c", c=64)
                        nc.sync.dma_start(scr_v[:, g0:g0 + 8],
                                          coll_i0[:, g0:g0 + 8])
                    nc.scalar.activation(coll_sm[:, c:c + 1], mx[:, 0:1],
                                         A.Copy)

                nc.sync.dma_start(oidx2_d[:], coll_i2[:])
                nc.sync.dma_start(osmax_d[:], coll_sm[:])
                wrapped = p2b.tile([128, 512], u16, tag="wrapped", bufs=1,
                                   name="wrapped")
                # slots 64*vg..64*vg+64 <=> chunks 8*vg..8*vg+8 (vecs 1024*vg..)
                srcv = idx0_scr[:].rearrange("(q r c) -> r c q", q=8, r=16,
                                             c=64)
                for g in range(8):
                    dst = wrapped[16 * g:16 * (g + 1), :].rearrange(
                        "r (c q) -> r c q", q=8)
                    nc.sync.dma_start(dst, srcv)
                nc.gpsimd.ap_gather(q0T[:], embT[:],
                                    wrapped[:].bitcast(i16), channels=128,
                                    num_elems=1024, d=1, num_idxs=8192)

                # quantized -> decoder input (padded, reuse xrf slots)
                q0pad = []
                for i in range(2):
                    t = mp.tile([128, 4356], DEC_DT, tag="xrf", bufs=2,
                                name=f"q0pad{i}")
                    _memset_rings66(nc, t[:])
                    tv = t[:].rearrange("p (h w) -> p h w", w=66)
                    qv = q0T[:, 4096 * i:4096 * (i + 1)].rearrange(
                        "p (h w) -> p h w", w=64)
                    for q in range(4):
                        nc.scalar.activation(
                            tv[:, 1 + 16 * q:1 + 16 * (q + 1), 1:65],
                            qv[:, 16 * q:16 * (q + 1), :], A.Copy)
                    q0pad.append(t)

            # ============ P3a: d_w1 conv + decoder res stack ============
            with tc.tile_pool(name="p3a", bufs=1) as p3a, \
                 tc.tile_pool(name="pp3a", bufs=7, space="PSUM") as pp3a:
                if DEC_DT != f32:
                    dw1r = p3a.tile([128, 9 * 128], DEC_DT, tag="dw1r",
                                    bufs=1, name="dw1r")
                    nc.scalar.activation(dw1r[:], dw1[:], A.Copy)
                    dwar = p3a.tile([128, 2 * 9 * 32], DEC_DT, tag="dwar",
                                    bufs=1, name="dwar")
                    nc.scalar.activation(dwar[:], dwa[:], A.Copy)
                else:
                    dw1r, dwar = dw1, dwa
                for i in range(2):
                    _memset_rings66(nc, xst[i][:])
                    yv = xst[i][:].rearrange("p (h w) -> p h w", w=66)

                    def emitd(bank, ps, yv=yv):
                        y0 = 8 * bank
                        psv = ps[:].rearrange("p (a b) -> p a b", b=64)
                        nc.scalar.activation(yv[:, 1 + y0:9 + y0, 1:65],
                                             psv, A.Identity, bias=db1[:])
                    _conv3x3(nc, pp3a, q0pad[i][:],
                             lambda p: dw1r[:, 128 * p:128 * (p + 1)],
                             128, 128, DEC_DT, emitd, f"dw1_{i}")
                res_stack(p3a, pp3a, xst, dwar, dwb, dg1, db1r, dg2, db2r,
                          DEC_DT, cc_i32, cc_o32, cc_i128, cc_o128, "d")
                yrf = []
                for i in range(2):
                    t = mp.tile([128, 4356], DEC_DT, tag="xrf", bufs=2,
                                name=f"dxrf{i}")
                    for q in range(4):
                        nc.scalar.activation(t[:, 1089 * q:1089 * (q + 1)],
                                             xst[i][:, 1089 * q:1089 * (q + 1)],
                                             A.Relu)
                    yrf.append(t)

            # ============ P3b: dt1 + dt2 ============
            with tc.tile_pool(name="p3b", bufs=1) as p3b, \
                 tc.tile_pool(name="pp3b", bufs=8, space="PSUM") as pp3b:
                dt1w = p3b.tile_from(dt1w_d[:])
                dt1b = p3b.tile_from(dt1b_d[:])
                dt2w = p3b.tile_from(dt2w_d[:])
                if DEC_DT != f32:
                    dt1wr = p3b.tile([128, 16 * 128], DEC_DT, tag="dt1wr",
                                     bufs=1, name="dt1wr")
                    nc.scalar.activation(dt1wr[:], dt1w[:], A.Copy)
                    dt2wr = p3b.tile([128, 24], DEC_DT, tag="dt2wr",
                                     bufs=1, name="dt2wr")
                    nc.scalar.activation(dt2wr[:], dt2w[:], A.Copy)
                else:
                    dt1wr, dt2wr = dt1w, dt2w
                for i in range(2):
                    X2 = p3b.tile([128, 16900], DEC_DT, tag="X2", bufs=1,
                                  name=f"X2_{i}")
                    _memset_rings130(nc, X2[:])
                    X2v = X2[:].rearrange("p (h w) -> p h w", w=130)
                    yv = yrf[i][:].rearrange("p (h w) -> p h w", w=66)
                    for a in range(2):
                        for b in range(2):
                            for bank in range(8):
                                u0 = 8 * bank
                                ps = pp3b.tile([128, 512], f32, tag="ps",
                                               name=f"t1_{i}_{a}{b}_{bank}")
                                k = 0
                                for t_i, (ky, sy) in enumerate(TAPS[a]):
                                    for s_i, (kx, sx) in enumerate(TAPS[b]):
                                        idx = ((a * 2 + b) * 2 + t_i) * 2 + s_i
                                        rhs = yv[:, 1 + u0 - sy:9 + u0 - sy,
                                                 1 - sx:65 - sx]
                                        _mm(nc, ps[:],
                                            dt1wr[:, 128 * idx:128 * (idx + 1)],
                                            rhs, DEC_DT, k == 0, k == 3)
                                        k += 1
                                psv = ps[:].rearrange("p (a b) -> p a b", b=64)
                                r0 = 1 + 2 * u0 + a
                                nc.scalar.activation(
                                    X2v[0:64, r0:r0 + 16:2,
                                        1 + b:129 + b:2],
                                    psv[0:64], A.Relu, bias=dt1b[0:64, :])
                                nc.vector.tensor_scalar(
                                    X2v[64:128, r0:r0 + 16:2, b:128 + b:2],
                                    psv[64:128], dt1b[64:128, :], 0.0,
                                    op0=mybir.AluOpType.add,
                                    op1=mybir.AluOpType.max)
                    for ch in range(32):
                        u0 = 4 * ch
                        ps2 = pp3b.tile([4, 512], f32, tag="ps",
                                        name=f"t2_{i}_{ch}")
                        for syi, sy in enumerate((-1, 0, 1)):
                            r0 = 1 + u0 - sy
                            rhs = X2v[0:128, r0:r0 + 4, 1:129]
                            _mm(nc, ps2[:], dt2wr[:, 8 * syi:8 * syi + 4],
                                rhs, DEC_DT, syi == 0, False)
                            rhs2 = X2v[0:64, r0:r0 + 4, 0:128]
                            _mm(nc, ps2[:],
                                dt2wr[0:64, 8 * syi + 4:8 * syi + 8],
                                rhs2, DEC_DT, False, syi == 2)
                        yst = p3b.tile([4, 512], f32, tag="yst", bufs=3,
                                       name=f"yst_{i}_{ch}")
                        nc.vector.tensor_copy(yst[:], ps2[:])
                        nc.sync.dma_start(
                            outy_d[i:i + 1, :, 512 * ch:512 * (ch + 1)],
                            yst[:])
    nc.compile()
    return nc


def _prep_inputs(inp):
    F = np.float32
    g = {k: np.ascontiguousarray(np.asarray(v), dtype=F) for k, v in inp.items()
         if k != 'emb'}
    g['emb'] = np.ascontiguousarray(np.asarray(inp['emb']), dtype=F)
    x = g['x']

    shared = {}
    shared['w1'] = np.concatenate([g['p_w1'].reshape(64, 16).T] * 2, axis=1)
    shared['b1'] = np.concatenate([g['p_b1']] * 2)[:, None]

    w2 = g['p_w2']  # [128, 64, 4, 4]
    w2p = np.zeros((128, 8, 128), F)
    for r in range(8):
        ky, kxp = r // 2, r % 2
        w2p[0:64, r] = w2[:, :, ky, 2 * kxp].T
        w2p[64:128, r] = w2[:, :, ky, 2 * kxp + 1].T
    shared['w2p'] = w2p.reshape(128, 1024)
    shared['b2'] = g['p_b2'][:, None]

    def conv9(w):  # [Cout, Cin, 3, 3] -> [Cin, 9*Cout]
        Cout, Cin = w.shape[0], w.shape[1]
        out = np.zeros((Cin, 9, Cout), F)
        for p, (ky, kx) in enumerate(POS9):
            out[:, p] = w[:, :, ky, kx].T
        return out.reshape(Cin, 9 * Cout)

    shared['w3'] = conv9(g['p_w3'])
    shared['b3'] = g['p_b3'][:, None]

    def resw(wa, wb):
        ewa = np.zeros((128, 2, 9, 32), F)
        for blk in range(2):
            ewa[:, blk] = conv9(wa[blk]).reshape(128, 9, 32)
        ewb = np.zeros((32, 2, 128), F)
        for blk in range(2):
            ewb[:, blk] = wb[blk][:, :, 0, 0].T
        return ewa.reshape(128, 576), ewb.reshape(32, 256)

    shared['ewa'], shared['ewb'] = resw(g['pr_wa'], g['pr_wb'])
    shared['eg1'] = g['pr_g1'].T.copy()
    shared['eb1'] = g['pr_b1'].T.copy()
    shared['eg2'] = g['pr_g2'].T.copy()
    shared['eb2'] = g['pr_b2'].T.copy()
    shared['prew'] = g['pre_w'][:, :, 0, 0].T.copy()
    shared['preb'] = g['pre_b'][:, None]
    emb = g['emb']
    shared['embT'] = emb.T.copy()
    shared['e2neg'] = (-0.5 * (emb.astype(F) ** 2).sum(1))[None, :].astype(F)
    shared['dw1'] = conv9(g['d_w1'])
    shared['db1'] = g['d_b1'][:, None]
    shared['dwa'], shared['dwb'] = resw(g['dr_wa'], g['dr_wb'])
    shared['dg1'] = g['dr_g1'].T.copy()
    shared['db1r'] = g['dr_b1'].T.copy()
    shared['dg2'] = g['dr_g2'].T.copy()
    shared['db2r'] = g['dr_b2'].T.copy()

    wt1 = g['dt1_w']  # [128, 64, 4, 4] (Cin, Cout, kh, kw)
    dt1w = np.zeros((128, 16, 128), F)
    for a in range(2):
        for b in range(2):
            for t_i, (ky, sy) in enumerate(TAPS[a]):
                for s_i, (kx, sx) in enumerate(TAPS[b]):
                    idx = ((a * 2 + b) * 2 + t_i) * 2 + s_i
                    dt1w[:, idx, 0:64] = wt1[:, :, ky, kx]
                    dt1w[:, idx, 64:128] = wt1[:, :, ky, kx]
    shared['dt1w'] = dt1w.reshape(128, 2048)
    shared['dt1b'] = np.concatenate([g['dt1_b']] * 2)[:, None]

    wt2 = g['dt2_w']  # [64, 1, 4, 4]
    ky_of = {0: {0: 1, 1: 3}, 1: {-1: 0, 0: 2}}  # [a][sy] -> ky
    kx_of = {0: {0: 1, 1: 3}, 1: {-1: 0, 0: 2}}
    dt2w = np.zeros((128, 6, 4), F)
    for syi, sy in enumerate((-1, 0, 1)):
        for a in range(2):
            if sy not in ky_of[a]:
                continue
            ky = ky_of[a][sy]
            for b in range(2):
                m = 2 * a + b
                # pair round: rows 0-63 sx=0, rows 64-127 sx=-1
                if 0 in kx_of[b]:
                    dt2w[0:64, 2 * syi, m] = wt2[:, 0, ky, kx_of[b][0]]
                if -1 in kx_of[b]:
                    dt2w[64:128, 2 * syi, m] = wt2[:, 0, ky, kx_of[b][-1]]
                # solo round: rows 0-63 sx=+1
                if 1 in kx_of[b]:
                    dt2w[0:64, 2 * syi + 1, m] = wt2[:, 0, ky, kx_of[b][1]]
    shared['dt2w'] = dt2w.reshape(128, 24)

    in_maps = []
    for core in range(NCORES):
        m = dict(shared)
        cols = np.zeros((16, 2, 128, 128), F)
        for i in range(2):
            img = x[2 * core + i, 0]
            xp = np.zeros((258, 258), F)
            xp[1:257, 1:257] = img
            for ky in range(4):
                for kx in range(4):
                    cols[ky * 4 + kx, i] = xp[ky:ky + 256:2, kx:kx + 256:2][:128, :128]
        m['xcol'] = cols.reshape(16, 2 * 16384)
        in_maps.append(m)
    return in_maps, g


_NC_CACHE = {}


def kernel(**inputs):
    in_maps, g = _prep_inputs(inputs)
    if 'nc' not in _NC_CACHE:
        _NC_CACHE['nc'] = _build()
    nc = _NC_CACHE['nc']
    res = run_bass_kernel_spmd(nc, in_maps, list(range(NCORES)))
    results = res.results

    F = np.float32
    x_recon = np.zeros((16, 1, 256, 256), F)
    dt2b = float(g['dt2_b'][0])
    tot = 0.0
    idx2_all = []
    for core in range(NCORES):
        r = results[core]
        y = r['out_y'].reshape(2, 4, 128, 128)
        for i in range(2):
            for a in range(2):
                for b in range(2):
                    x_recon[2 * core + i, 0, a::2, b::2] = y[i, 2 * a + b]
        tot += float(r['out_zsq'].astype(np.float64).sum())
        tot -= 2.0 * float(r['out_smax'].astype(np.float64).sum())
        idx2_all.append(r['out_idx2'].T.reshape(-1))
    x_recon += F(dt2b)

    e_latent = tot / (16 * 4096 * 128)
    loss = F(0.25 * e_latent)

    counts = np.bincount(np.concatenate(idx2_all).astype(np.int64),
                         minlength=1024).astype(np.float64)
    avg = counts / (16 * 4096)
    perp = F(np.exp(-np.sum(avg * np.log(avg + 1e-10))))
    return loss, x_recon, perp


# revision 31
# speedup vs baseline: 1.0598x; 1.0309x over previous
"""BeamVQ (VQ-VAE fwd) Trainium2 kernel — 8-core batch-parallel.

Strategy: shard batch 16 -> 8 cores x 2 images. Convs as shift-matmuls on PE
(channels on partitions, PSUM accumulation). Training-mode BN via per-channel
sum/sumsq with a tiny in-kernel AllReduce per BN layer (8 total). VQ: PE fp32
distance scores s = z.e - |e|^2/2, DVE max/max_index top-8, gpsimd ap_gather
codebook lookup. ConvTranspose via output-parity decomposition. Loss /
perplexity partial sums are finished on the host.

Precision: encoder + pre-VQ + distance matmuls in true fp32 (top-1 gaps go
down to 5e-5; fp32r's ~1e-5 operand rounding flips argmins and corrupts the
reconstruction by O(1)). Decoder matmuls run in fp32r (4x faster PE streams;
weights/activations are rounded to f32r by their ACT producers as the BIR
verifier requires) giving recon absmax ~1.4e-3 on scale 3.7; set DEC_DT = f32
to get absmax ~6e-6 at +~550us.

Measured (8 trn2 cores via axon/PJRT): recon absmax 1.38e-3 (rel 3.8e-4),
loss rel 3.5e-7, perplexity rel 5.5e-6 vs the fp32 CPU reference.
Cost-model timeline estimate ~1.33 ms end-to-end (excl. AllReduce latency).
Filler matmuls in the BN/collective valleys keep the PE clock warm; dup-half
padded writes are split ACT(main)/DVE(shifted dup) to balance engines.
"""
import numpy as np

import concourse.bass as bass
import concourse.mybir as mybir
from concourse import bacc, tile
from concourse.bass_utils import run_bass_kernel_spmd

f32 = mybir.dt.float32
f32r = mybir.dt.float32r
u16 = mybir.dt.uint16
i16 = mybir.dt.int16

NCORES = 8
ENC_DT = f32    # z-path matmul dtype (must stay fp32)
DEC_DT = f32r   # decoder matmul dtype (operands produced as rounded f32r)

POS9 = [(ky, kx) for ky in range(3) for kx in range(3)]
# transposed-conv k4 s2 p1 parity taps: parity -> [(k, shift)]
TAPS = {0: [(1, 0), (3, 1)], 1: [(0, -1), (2, 0)]}

A = mybir.ActivationFunctionType


def _mm(nc, ps, lhsT, rhs, dt, start, stop):
    # operands must already carry dtype `dt` (f32r data must be produced
    # rounded — the BIR verifier rejects plain bitcasts of f32 data)
    nc.tensor.matmul(ps, lhsT, rhs, start=start, stop=stop)


def _conv3x3(nc, pp, src, wsl, Cin, Cout, dt, emit, name):
    """src: padded-66 AP [>=Cin, 4356]; wsl(pos) -> lhsT [Cin, Cout];
    emit(bank, ps) consumes psum [Cout, 512] for out rows 8*bank..8*bank+8."""
    v = src.rearrange("p (h w) -> p h w", w=66)
    for bank in range(8):
        y0 = 8 * bank
        ps = pp.tile([Cout, 512], f32, tag="ps", name=f"{name}_ps{bank}")
        for pos, (ky, kx) in enumerate(POS9):
            rhs = v[0:Cin, y0 + ky:y0 + ky + 8, kx:kx + 64]
            _mm(nc, ps[:], wsl(pos), rhs, dt, pos == 0, pos == 8)
        emit(bank, ps)


def _memset_rings66(nc, t):
    v = t.bitcast(mybir.dt.uint32).rearrange("p (h w) -> p h w", w=66)
    nc.gpsimd.memset(v[:, 0:66:65, :], 0)
    nc.gpsimd.memset(v[:, :, 0:1], 0)
    nc.gpsimd.memset(v[:, :, 65:66], 0)


def _memset_rings130(nc, t):
    v = t.bitcast(mybir.dt.uint32).rearrange("p (h w) -> p h w", w=130)
    nc.gpsimd.memset(v[:, 0:130:129, :], 0)
    nc.gpsimd.memset(v[:, :, 0:1], 0)
    nc.gpsimd.memset(v[:, :, 128:130], 0)


def _build(with_cc=True):
    nc = bacc.Bacc("TRN2", target_bir_lowering=False, debug=False,
                   enable_asserts=False,
                   num_devices=NCORES if with_cc else 1)

    def din(name, shape, dt=f32):
        return nc.dram_tensor(name, shape, dt, kind="ExternalInput")

    def dout(name, shape, dt=f32):
        return nc.dram_tensor(name, shape, dt, kind="ExternalOutput")

    xcol_d = din("xcol", [16, 2 * 16384])
    w1_d = din("w1", [16, 128])
    b1_d = din("b1", [128, 1])
    w2p_d = din("w2p", [128, 8 * 128])
    b2_d = din("b2", [128, 1])
    w3_d = din("w3", [128, 9 * 128])
    b3_d = din("b3", [128, 1])
    ewa_d = din("ewa", [128, 2 * 9 * 32])
    ewb_d = din("ewb", [32, 2 * 128])
    eg1_d = din("eg1", [32, 2])
    eb1_d = din("eb1", [32, 2])
    eg2_d = din("eg2", [128, 2])
    eb2_d = din("eb2", [128, 2])
    prew_d = din("prew", [128, 128])
    preb_d = din("preb", [128, 1])
    embT_d = din("embT", [128, 1024])
    e2neg_d = din("e2neg", [1, 1024])
    dw1_d = din("dw1", [128, 9 * 128])
    db1_d = din("db1", [128, 1])
    dwa_d = din("dwa", [128, 2 * 9 * 32])
    dwb_d = din("dwb", [32, 2 * 128])
    dg1_d = din("dg1", [32, 2])
    db1r_d = din("db1r", [32, 2])
    dg2_d = din("dg2", [128, 2])
    db2r_d = din("db2r", [128, 2])
    dt1w_d = din("dt1w", [128, 16 * 128])
    dt1b_d = din("dt1b", [128, 1])
    dt2w_d = din("dt2w", [128, 24])

    outy_d = dout("out_y", [2, 4, 16384])
    osmax_d = dout("out_smax", [128, 64])
    oidx2_d = dout("out_idx2", [128, 64], u16)
    ozsq_d = dout("out_zsq", [128, 1])

    with tile.TileContext(nc) as tc:
        with tc.tile_pool(name="wpool", bufs=1) as wp, \
             tc.tile_pool(name="main", bufs=1) as mp, \
             tc.tile_pool(name="small", bufs=2) as sp, \
             tc.tile_pool(name="dram", bufs=1, space="DRAM") as dr:

            w1 = wp.tile_from(w1_d[:])
            b1 = wp.tile_from(b1_d[:])
            w2p = wp.tile_from(w2p_d[:])
            b2 = wp.tile_from(b2_d[:])
            w3 = wp.tile_from(w3_d[:])
            b3 = wp.tile_from(b3_d[:])
            ewa = wp.tile_from(ewa_d[:])
            ewb = wp.tile_from(ewb_d[:])
            eg1 = wp.tile_from(eg1_d[:])
            eb1 = wp.tile_from(eb1_d[:])
            eg2 = wp.tile_from(eg2_d[:])
            eb2 = wp.tile_from(eb2_d[:])
            prew = wp.tile_from(prew_d[:])
            preb = wp.tile_from(preb_d[:])
            embT = wp.tile_from(embT_d[:])
            dw1 = wp.tile_from(dw1_d[:])
            db1 = wp.tile_from(db1_d[:])
            dwa = wp.tile_from(dwa_d[:])
            dwb = wp.tile_from(dwb_d[:])
            dg1 = wp.tile_from(dg1_d[:])
            db1r = wp.tile_from(db1r_d[:])
            dg2 = wp.tile_from(dg2_d[:])
            db2r = wp.tile_from(db2r_d[:])

            cc_i32 = dr.tile([32, 2], f32, name="cc_i32")
            cc_o32 = dr.tile([32, 2], f32, name="cc_o32")
            cc_i128 = dr.tile([128, 2], f32, name="cc_i128")
            cc_o128 = dr.tile([128, 2], f32, name="cc_o128")
            idx0_scr = dr.tile([8192], u16, name="idx0_scr")

            eps_t = sp.tile([128, 1], f32, tag="eps", bufs=1, name="eps_t")
            nc.gpsimd.memset(eps_t[:], 1e-5)
            coll_i0 = sp.tile([128, 64], u16, tag="ci0", bufs=1, name="coll_i0")
            coll_i2 = sp.tile([128, 64], u16, tag="ci2", bufs=1, name="coll_i2")
            coll_sm = sp.tile([128, 64], f32, tag="csm", bufs=1, name="coll_sm")

            def bn_cc(raw, C, g_ap, b_ap, cci, cco, name, pp=None):
                """raw: SBUF AP [C, 8192] of raw conv outputs (both images).
                Returns (a, c) coefficient APs [C, 1]."""
                st6 = sp.tile([C, 96], f32, tag=f"st6_{C}", name=f"{name}_st6")
                for i in range(16):
                    nc.vector.bn_stats(st6[:, 6 * i:6 * i + 6],
                                       raw[:, 512 * i:512 * (i + 1)])
                mv = sp.tile([C, 2], f32, tag=f"mv_{C}", name=f"{name}_mv")
                nc.vector.bn_aggr(mv[:], st6[:])
                m2 = sp.tile([C, 1], f32, tag=f"m2_{C}", name=f"{name}_m2")
                nc.vector.tensor_tensor(m2[:], mv[:, 0:1], mv[:, 0:1],
                                        op=mybir.AluOpType.mult)
                ex2 = sp.tile([C, 1], f32, tag=f"ex2_{C}", name=f"{name}_ex2")
                nc.vector.tensor_tensor(ex2[:], mv[:, 1:2], m2[:],
                                        op=mybir.AluOpType.add)
                csb = sp.tile([C, 2], f32, tag=f"csb_{C}", name=f"{name}_csb")
                nc.vector.tensor_scalar_mul(csb[:, 0:1], mv[:, 0:1], 8192.0)
                nc.vector.tensor_scalar_mul(csb[:, 1:2], ex2[:], 8192.0)
                nc.sync.dma_start(cci[:], csb[:])
                if with_cc:
                    nc.gpsimd.collective_compute(
                        "AllReduce", mybir.AluOpType.add,
                        replica_groups=[list(range(NCORES))],
                        ins=[cci.opt()], outs=[cco.opt()])
                else:
                    nc.sync.dma_start(cco[:], cci[:])
                if pp is not None:
                    fill = pp.tile([128, 512], f32, tag="fill", bufs=1,
                                   name=f"{name}_fill")
                    for k in range(24):
                        nc.tensor.matmul(fill[:], embT[:, 0:128],
                                         embT[:, 0:512], start=True,
                                         stop=True)
                gsb = sp.tile([C, 2], f32, tag=f"gsb_{C}", name=f"{name}_gsb")
                nc.sync.dma_start(gsb[:], cco[:])
                gm = sp.tile([C, 1], f32, tag=f"gm_{C}", name=f"{name}_gm")
                nc.vector.tensor_scalar_mul(gm[:], gsb[:, 0:1], 1.0 / 65536.0)
                gex2 = sp.tile([C, 1], f32, tag=f"gex2_{C}", name=f"{name}_gex2")
                nc.vector.tensor_scalar_mul(gex2[:], gsb[:, 1:2], 1.0 / 65536.0)
                gm2 = sp.tile([C, 1], f32, tag=f"gm2_{C}", name=f"{name}_gm2")
                nc.vector.tensor_tensor(gm2[:], gm[:], gm[:],
                                        op=mybir.AluOpType.mult)
                gvar = sp.tile([C, 1], f32, tag=f"gvar_{C}", name=f"{name}_gvar")
                nc.vector.tensor_tensor(gvar[:], gex2[:], gm2[:],
                                        op=mybir.AluOpType.subtract)
                sd = sp.tile([C, 1], f32, tag=f"sd_{C}", name=f"{name}_sd")
                nc.scalar.activation(sd[:], gvar[:], A.Sqrt, bias=eps_t[0:C, :])
                inv = sp.tile([C, 1], f32, tag=f"inv_{C}", name=f"{name}_inv")
                nc.vector.reciprocal(inv[:], sd[:])
                a_t = sp.tile([C, 1], f32, tag=f"a_{C}", name=f"{name}_a")
                nc.vector.tensor_tensor(a_t[:], g_ap, inv[:],
                                        op=mybir.AluOpType.mult)
                am = sp.tile([C, 1], f32, tag=f"am_{C}", name=f"{name}_am")
                nc.vector.tensor_tensor(am[:], a_t[:], gm[:],
                                        op=mybir.AluOpType.mult)
                c_t = sp.tile([C, 1], f32, tag=f"c_{C}", name=f"{name}_c")
                nc.vector.tensor_tensor(c_t[:], b_ap, am[:],
                                        op=mybir.AluOpType.subtract)
                return a_t[:], c_t[:]

            def res_stack(P, pp, xs, wa, wb, g1s, b1s, g2s, b2s, dt, cci32,
                          cco32, cci128, cco128, pre):
                """xs: list of 2 padded-66 x-state tile APs. Returns xs."""
                for blk in range(2):
                    xr = []
                    for i in range(2):
                        t = P.tile([128, 4356], dt, tag="xr", bufs=1,
                                   name=f"{pre}xr{blk}_{i}")
                        for q in range(4):
                            nc.scalar.activation(t[:, 1089 * q:1089 * (q + 1)],
                                                 xs[i][:, 1089 * q:1089 * (q + 1)],
                                                 A.Relu)
                        xr.append(t)
                    rawa = P.tile([32, 8192], f32, tag="rawa", bufs=1,
                                  name=f"{pre}rawa{blk}")

                    for i in range(2):
                        def emit_a(bank, ps, i=i):
                            off = 4096 * i + 512 * bank
                            nc.scalar.activation(rawa[:, off:off + 512], ps[:],
                                                 A.Copy)
                        _conv3x3(nc, pp, xr[i][:],
                                 lambda p: wa[:, (9 * blk + p) * 32:
                                              (9 * blk + p) * 32 + 32],
                                 128, 32, dt, emit_a, f"{pre}ca{blk}_{i}")
                    a1, c1 = bn_cc(rawa[:], 32, g1s[:, blk:blk + 1],
                                   b1s[:, blk:blk + 1], cci32, cco32,
                                   f"{pre}bn1_{blk}", pp=pp)
                    for q in range(16):
                        sl = slice(512 * q, 512 * (q + 1))
                        nc.scalar.activation(rawa[:, sl], rawa[:, sl], A.Relu,
                                             bias=c1, scale=a1)
                    rawb = P.tile([128, 8192], f32, tag="rawb", bufs=1,
                                  name=f"{pre}rawb{blk}")
                    for i in range(2):
                        for bank in range(8):
                            off = 4096 * i + 512 * bank
                            ps = pp.tile([128, 512], f32, tag="ps",
                                         name=f"{pre}cb{blk}_{i}_{bank}")
                            _mm(nc, ps[:],
                                wb[:, 128 * blk:128 * blk + 128],
                                rawa[0:32, off:off + 512], f32, True, True)
                            nc.scalar.activation(rawb[:, off:off + 512],
                                                 ps[:], A.Copy)
                    a2, c2 = bn_cc(rawb[:], 128, g2s[:, blk:blk + 1],
                                   b2s[:, blk:blk + 1], cci128, cco128,
                                   f"{pre}bn2_{blk}", pp=pp)
                    for q in range(16):
                        sl = slice(512 * q, 512 * (q + 1))
                        nc.scalar.activation(rawb[:, sl], rawb[:, sl],
                                             A.Identity, bias=c2, scale=a2)
                    for i in range(2):
                        xv = xs[i][:].rearrange("p (h w) -> p h w", w=66)
                        xint = xv[:, 1:65, 1:65]
                        rv = rawb[:, 4096 * i:4096 * (i + 1)].rearrange(
                            "p (h w) -> p h w", w=64)
                        nc.vector.tensor_tensor(xint, xint, rv,
                                                op=mybir.AluOpType.add)
                return xs

            # ============ P1: conv1 + conv2 + conv3 ============
            xst = []
            for i in range(2):
                t = mp.tile([128, 4356], f32, tag="xst", bufs=2,
                            name=f"xst{i}")
                xst.append(t)
            with tc.tile_pool(name="p1", bufs=1) as p1, \
                 tc.tile_pool(name="pp1", bufs=8, space="PSUM") as pp1:
                for i in range(2):
                    X1 = p1.tile([128, 16900], f32, tag="X1", bufs=1,
                                 name=f"X1_{i}")
                    _memset_rings130(nc, X1[:])
                    X1v = X1[:].rearrange("p (h w) -> p h w", w=130)
                    for c in range(32):
                        stg = p1.tile([16, 512], f32, tag="c1s", bufs=3,
                                      name=f"c1s_{i}_{c}")
                        nc.gpsimd.dma_start(
                            stg[:], xcol_d[:, 16384 * i + 512 * c:
                                           16384 * i + 512 * (c + 1)])
                        ps = pp1.tile([128, 512], f32, tag="ps",
                                      name=f"c1ps_{i}_{c}")
                        _mm(nc, ps[:], w1[:], stg[:], ENC_DT, True, True)
                        psv = ps[:].rearrange("p (a b) -> p a b", b=128)
                        nc.scalar.activation(
                            X1v[0:64, 1 + 4 * c:5 + 4 * c, 1:129],
                            psv[0:64], A.Relu, bias=b1[0:64, :])
                        nc.scalar.activation(
                            X1v[64:128, 1 + 4 * c:5 + 4 * c, 0:128],
                            psv[64:128], A.Relu, bias=b1[64:128, :])
                    c2o = p1.tile([128, 4356], f32, tag="c2o", bufs=1,
                                  name=f"c2o_{i}")
                    _memset_rings66(nc, c2o[:])
                    c2ov = c2o[:].rearrange("p (h w) -> p h w", w=66)
                    for bank in range(8):
                        y0 = 8 * bank
                        ps = pp1.tile([128, 512], f32, tag="ps",
                                      name=f"c2ps_{i}_{bank}")
                        for r in range(8):
                            ky, kxp = r // 2, r % 2
                            rhs = X1v[:, 2 * y0 + ky:2 * y0 + ky + 15:2,
                                      2 * kxp:2 * kxp + 127:2]
                            _mm(nc, ps[:], w2p[:, 128 * r:128 * (r + 1)],
                                rhs, ENC_DT, r == 0, r == 7)
                        psv = ps[:].rearrange("p (a b) -> p a b", b=64)
                        nc.scalar.activation(c2ov[:, 1 + y0:9 + y0, 1:65],
                                             psv, A.Relu, bias=b2[:])
                    _memset_rings66(nc, xst[i][:])
                    xiv = xst[i][:].rearrange("p (h w) -> p h w", w=66)

                    def emit3(bank, ps, xiv=xiv):
                        y0 = 8 * bank
                        psv = ps[:].rearrange("p (a b) -> p a b", b=64)
                        nc.scalar.activation(xiv[:, 1 + y0:9 + y0, 1:65],
                                             psv, A.Identity, bias=b3[:])
                    _conv3x3(nc, pp1, c2o[:],
                             lambda p: w3[:, 128 * p:128 * (p + 1)],
                             128, 128, ENC_DT, emit3, f"c3_{i}")

            # ============ P2a: encoder res stack ============
            with tc.tile_pool(name="p2a", bufs=1) as p2a, \
                 tc.tile_pool(name="pp2a", bufs=7, space="PSUM") as pp2a:
                res_stack(p2a, pp2a, xst, ewa, ewb, eg1, eb1, eg2, eb2,
                          ENC_DT, cc_i32, cc_o32, cc_i128, cc_o128, "e")
                xrf = []
                for i in range(2):
                    t = mp.tile([128, 4356], f32, tag="xrf", bufs=2,
                                name=f"exrf{i}")
                    for q in range(4):
                        nc.scalar.activation(t[:, 1089 * q:1089 * (q + 1)],
                                             xst[i][:, 1089 * q:1089 * (q + 1)],
                                             A.Relu)
                    xrf.append(t)

            # ============ P2b: pre-VQ conv + VQ ============
            with tc.tile_pool(name="p2b", bufs=1) as p2b, \
                 tc.tile_pool(name="pp2b", bufs=2, space="PSUM") as pp2b:
                z = p2b.tile([128, 8192], f32, tag="z", bufs=1, name="z")
                for i in range(2):
                    xv = xrf[i][:].rearrange("p (h w) -> p h w", w=66)
                    for bank in range(8):
                        y0 = 8 * bank
                        ps = pp2b.tile([128, 512], f32, tag="pvps", bufs=2,
                                       name=f"pv_{i}_{bank}")
                        _mm(nc, ps[:], prew[:], xv[:, 1 + y0:9 + y0, 1:65],
                            ENC_DT, True, True)
                        off = 4096 * i + 512 * bank
                        nc.scalar.activation(z[:, off:off + 512], ps[:],
                                             A.Identity, bias=preb[:])
                q0T = p2b.tile([128, 8192], f32, tag="q0T", bufs=1, name="q0T")
                zsq = sp.tile([128, 1], f32, tag="zsq", bufs=1, name="zsq")
                nc.scalar.activation(q0T[:], z[:], A.Square, accum_out=zsq[:])
                nc.sync.dma_start(ozsq_d[:], zsq[:])

                e2neg_t = p2b.tile_from(e2neg_d[:])
                e2rep = p2b.tile([128, 1024], f32, tag="e2rep", bufs=1,
                                 name="e2rep")
                nc.gpsimd.partition_broadcast(e2rep[:], e2neg_t[:])

                for c in range(64):
                    ps = pp2b.tile([128, 1024], f32, tag="vqps", bufs=3,
                                   name=f"vqps{c}")
                    for h in range(2):
                        _mm(nc, ps[:, 512 * h:512 * (h + 1)],
                            z[:, 128 * c:128 * (c + 1)],
                            embT[:, 512 * h:512 * (h + 1)], ENC_DT,
                            True, True)
                    s_t = p2b.tile([128, 1024], f32, tag="s", bufs=4,
                                   name=f"s{c}")
                    nc.scalar.activation(s_t[:, 0:512], ps[:, 0:512], A.Copy)
                    nc.scalar.activation(s_t[:, 512:1024], ps[:, 512:1024],
                                         A.Copy)
                    nc.gpsimd.tensor_tensor(s_t[:], s_t[:], e2rep[:],
                                            op=mybir.AluOpType.add)
                    mx = p2b.tile([128, 8], f32, tag="mx", bufs=3,
                                  name=f"mx{c}")
                    ix = p2b.tile([128, 8], u16, tag="ix", bufs=3,
                                  name=f"ix{c}")
                    nc.vector.max(mx[:], s_t[:])
                    nc.vector.max_index(ix[:], mx[:], s_t[:])
                    nc.gpsimd.tensor_copy(coll_i0[:, c:c + 1], ix[:, 0:1])
                    nc.gpsimd.tensor_copy(coll_i2[:, c:c + 1], ix[:, 2:3])
                    if c % 8 == 7:
                        g0 = c - 7
                        scr_v = idx0_scr[:].rearrange("(p c) -> p c", c=64)
                        nc.sync.dma_start(scr_v[:, g0:g0 + 8],
                                          coll_i0[:, g0:g0 + 8])
                    nc.scalar.activation(coll_sm[:, c:c + 1], mx[:, 0:1],
                                         A.Copy)

                vqfill = pp2b.tile([128, 512], f32, tag="pvps",
                                   name="vqfill")
                for k in range(20):
                    nc.tensor.matmul(vqfill[:], embT[:, 0:128],
                                     embT[:, 0:512], start=True, stop=True)
                nc.sync.dma_start(oidx2_d[:], coll_i2[:])
                nc.sync.dma_start(osmax_d[:], coll_sm[:])
                wrapped = p2b.tile([128, 512], u16, tag="wrapped", bufs=1,
                                   name="wrapped")
                # slots 64*vg..64*vg+64 <=> chunks 8*vg..8*vg+8 (vecs 1024*vg..)
                srcv = idx0_scr[:].rearrange("(q r c) -> r c q", q=8, r=16,
                                             c=64)
                for g in range(8):
                    dst = wrapped[16 * g:16 * (g + 1), :].rearrange(
                        "r (c q) -> r c q", q=8)
                    nc.sync.dma_start(dst, srcv)
                nc.gpsimd.ap_gather(q0T[:], embT[:],
                                    wrapped[:].bitcast(i16), channels=128,
                                    num_elems=1024, d=1, num_idxs=8192)

                # quantized -> decoder input (padded, reuse xrf slots)
                q0pad = []
                for i in range(2):
                    t = mp.tile([128, 4356], DEC_DT, tag="xrf", bufs=2,
                                name=f"q0pad{i}")
                    _memset_rings66(nc, t[:])
                    tv = t[:].rearrange("p (h w) -> p h w", w=66)
                    qv = q0T[:, 4096 * i:4096 * (i + 1)].rearrange(
                        "p (h w) -> p h w", w=64)
                    for q in range(4):
                        nc.scalar.activation(
                            tv[:, 1 + 16 * q:1 + 16 * (q + 1), 1:65],
                            qv[:, 16 * q:16 * (q + 1), :], A.Copy)
                    q0pad.append(t)

            # ============ P3a: d_w1 conv + decoder res stack ============
            with tc.tile_pool(name="p3a", bufs=1) as p3a, \
                 tc.tile_pool(name="pp3a", bufs=7, space="PSUM") as pp3a:
                if DEC_DT != f32:
                    dw1r = p3a.tile([128, 9 * 128], DEC_DT, tag="dw1r",
                                    bufs=1, name="dw1r")
                    nc.scalar.activation(dw1r[:], dw1[:], A.Copy)
                    dwar = p3a.tile([128, 2 * 9 * 32], DEC_DT, tag="dwar",
                                    bufs=1, name="dwar")
                    nc.scalar.activation(dwar[:], dwa[:], A.Copy)
                else:
                    dw1r, dwar = dw1, dwa
                for i in range(2):
                    _memset_rings66(nc, xst[i][:])
                    yv = xst[i][:].rearrange("p (h w) -> p h w", w=66)

                    def emitd(bank, ps, yv=yv):
                        y0 = 8 * bank
                        psv = ps[:].rearrange("p (a b) -> p a b", b=64)
                        nc.scalar.activation(yv[:, 1 + y0:9 + y0, 1:65],
                                             psv, A.Identity, bias=db1[:])
                    _conv3x3(nc, pp3a, q0pad[i][:],
                             lambda p: dw1r[:, 128 * p:128 * (p + 1)],
                             128, 128, DEC_DT, emitd, f"dw1_{i}")
                res_stack(p3a, pp3a, xst, dwar, dwb, dg1, db1r, dg2, db2r,
                          DEC_DT, cc_i32, cc_o32, cc_i128, cc_o128, "d")
                yrf = []
                for i in range(2):
                    t = mp.tile([128, 4356], DEC_DT, tag="xrf", bufs=2,
                                name=f"dxrf{i}")
                    for q in range(4):
                        nc.scalar.activation(t[:, 1089 * q:1089 * (q + 1)],
                                             xst[i][:, 1089 * q:1089 * (q + 1)],
                                             A.Relu)
                    yrf.append(t)

            # ============ P3b: dt1 + dt2 ============
            with tc.tile_pool(name="p3b", bufs=1) as p3b, \
                 tc.tile_pool(name="pp3b", bufs=8, space="PSUM") as pp3b:
                dt1w = p3b.tile_from(dt1w_d[:])
                dt1b = p3b.tile_from(dt1b_d[:])
                dt2w = p3b.tile_from(dt2w_d[:])
                if DEC_DT != f32:
                    dt1wr = p3b.tile([128, 16 * 128], DEC_DT, tag="dt1wr",
                                     bufs=1, name="dt1wr")
                    nc.scalar.activation(dt1wr[:], dt1w[:], A.Copy)
                    dt2wr = p3b.tile([128, 24], DEC_DT, tag="dt2wr",
                                     bufs=1, name="dt2wr")
                    nc.scalar.activation(dt2wr[:], dt2w[:], A.Copy)
                else:
                    dt1wr, dt2wr = dt1w, dt2w
                for i in range(2):
                    X2 = p3b.tile([128, 16900], DEC_DT, tag="X2", bufs=1,
                                  name=f"X2_{i}")
                    _memset_rings130(nc, X2[:])
                    X2v = X2[:].rearrange("p (h w) -> p h w", w=130)
                    yv = yrf[i][:].rearrange("p (h w) -> p h w", w=66)
                    for a in range(2):
                        for b in range(2):
                            for bank in range(8):
                                u0 = 8 * bank
                                ps = pp3b.tile([128, 512], f32, tag="ps",
                                               name=f"t1_{i}_{a}{b}_{bank}")
                                k = 0
                                for t_i, (ky, sy) in enumerate(TAPS[a]):
                                    for s_i, (kx, sx) in enumerate(TAPS[b]):
                                        idx = ((a * 2 + b) * 2 + t_i) * 2 + s_i
                                        rhs = yv[:, 1 + u0 - sy:9 + u0 - sy,
                                                 1 - sx:65 - sx]
                                        _mm(nc, ps[:],
                                            dt1wr[:, 128 * idx:128 * (idx + 1)],
                                            rhs, DEC_DT, k == 0, k == 3)
                                        k += 1
                                psv = ps[:].rearrange("p (a b) -> p a b", b=64)
                                r0 = 1 + 2 * u0 + a
                                nc.scalar.activation(
                                    X2v[0:64, r0:r0 + 16:2,
                                        1 + b:129 + b:2],
                                    psv[0:64], A.Relu, bias=dt1b[0:64, :])
                                nc.vector.tensor_scalar(
                                    X2v[64:128, r0:r0 + 16:2, b:128 + b:2],
                                    psv[64:128], dt1b[64:128, :], 0.0,
                                    op0=mybir.AluOpType.add,
                                    op1=mybir.AluOpType.max)
                    for ch in range(32):
                        u0 = 4 * ch
                        ps2 = pp3b.tile([4, 512], f32, tag="ps",
                                        name=f"t2_{i}_{ch}")
                        for syi, sy in enumerate((-1, 0, 1)):
                            r0 = 1 + u0 - sy
                            rhs = X2v[0:128, r0:r0 + 4, 1:129]
                            _mm(nc, ps2[:], dt2wr[:, 8 * syi:8 * syi + 4],
                                rhs, DEC_DT, syi == 0, False)
                            rhs2 = X2v[0:64, r0:r0 + 4, 0:128]
                            _mm(nc, ps2[:],
                                dt2wr[0:64, 8 * syi + 4:8 * syi + 8],
                                rhs2, DEC_DT, False, syi == 2)
                        yst = p3b.tile([4, 512], f32, tag="yst", bufs=3,
                                       name=f"yst_{i}_{ch}")
                        nc.vector.tensor_copy(yst[:], ps2[:])
                        nc.sync.dma_start(
                            outy_d[i:i + 1, :, 512 * ch:512 * (ch + 1)],
                            yst[:])
    nc.compile()
    return nc


def _prep_inputs(inp):
    F = np.float32
    g = {k: np.ascontiguousarray(np.asarray(v), dtype=F) for k, v in inp.items()
         if k != 'emb'}
    g['emb'] = np.ascontiguousarray(np.asarray(inp['emb']), dtype=F)
    x = g['x']

    shared = {}
    shared['w1'] = np.concatenate([g['p_w1'].reshape(64, 16).T] * 2, axis=1)
    shared['b1'] = np.concatenate([g['p_b1']] * 2)[:, None]

    w2 = g['p_w2']  # [128, 64, 4, 4]
    w2p = np.zeros((128, 8, 128), F)
    for r in range(8):
        ky, kxp = r // 2, r % 2
        w2p[0:64, r] = w2[:, :, ky, 2 * kxp].T
        w2p[64:128, r] = w2[:, :, ky, 2 * kxp + 1].T
    shared['w2p'] = w2p.reshape(128, 1024)
    shared['b2'] = g['p_b2'][:, None]

    def conv9(w):  # [Cout, Cin, 3, 3] -> [Cin, 9*Cout]
        Cout, Cin = w.shape[0], w.shape[1]
        out = np.zeros((Cin, 9, Cout), F)
        for p, (ky, kx) in enumerate(POS9):
            out[:, p] = w[:, :, ky, kx].T
        return out.reshape(Cin, 9 * Cout)

    shared['w3'] = conv9(g['p_w3'])
    shared['b3'] = g['p_b3'][:, None]

    def resw(wa, wb):
        ewa = np.zeros((128, 2, 9, 32), F)
        for blk in range(2):
            ewa[:, blk] = conv9(wa[blk]).reshape(128, 9, 32)
        ewb = np.zeros((32, 2, 128), F)
        for blk in range(2):
            ewb[:, blk] = wb[blk][:, :, 0, 0].T
        return ewa.reshape(128, 576), ewb.reshape(32, 256)

    shared['ewa'], shared['ewb'] = resw(g['pr_wa'], g['pr_wb'])
    shared['eg1'] = g['pr_g1'].T.copy()
    shared['eb1'] = g['pr_b1'].T.copy()
    shared['eg2'] = g['pr_g2'].T.copy()
    shared['eb2'] = g['pr_b2'].T.copy()
    shared['prew'] = g['pre_w'][:, :, 0, 0].T.copy()
    shared['preb'] = g['pre_b'][:, None]
    emb = g['emb']
    shared['embT'] = emb.T.copy()
    shared['e2neg'] = (-0.5 * (emb.astype(F) ** 2).sum(1))[None, :].astype(F)
    shared['dw1'] = conv9(g['d_w1'])
    shared['db1'] = g['d_b1'][:, None]
    shared['dwa'], shared['dwb'] = resw(g['dr_wa'], g['dr_wb'])
    shared['dg1'] = g['dr_g1'].T.copy()
    shared['db1r'] = g['dr_b1'].T.copy()
    shared['dg2'] = g['dr_g2'].T.copy()
    shared['db2r'] = g['dr_b2'].T.copy()

    wt1 = g['dt1_w']  # [128, 64, 4, 4] (Cin, Cout, kh, kw)
    dt1w = np.zeros((128, 16, 128), F)
    for a in range(2):
        for b in range(2):
            for t_i, (ky, sy) in enumerate(TAPS[a]):
                for s_i, (kx, sx) in enumerate(TAPS[b]):
                    idx = ((a * 2 + b) * 2 + t_i) * 2 + s_i
                    dt1w[:, idx, 0:64] = wt1[:, :, ky, kx]
                    dt1w[:, idx, 64:128] = wt1[:, :, ky, kx]
    shared['dt1w'] = dt1w.reshape(128, 2048)
    shared['dt1b'] = np.concatenate([g['dt1_b']] * 2)[:, None]

    wt2 = g['dt2_w']  # [64, 1, 4, 4]
    ky_of = {0: {0: 1, 1: 3}, 1: {-1: 0, 0: 2}}  # [a][sy] -> ky
    kx_of = {0: {0: 1, 1: 3}, 1: {-1: 0, 0: 2}}
    dt2w = np.zeros((128, 6, 4), F)
    for syi, sy in enumerate((-1, 0, 1)):
        for a in range(2):
            if sy not in ky_of[a]:
                continue
            ky = ky_of[a][sy]
            for b in range(2):
                m = 2 * a + b
                # pair round: rows 0-63 sx=0, rows 64-127 sx=-1
                if 0 in kx_of[b]:
                    dt2w[0:64, 2 * syi, m] = wt2[:, 0, ky, kx_of[b][0]]
                if -1 in kx_of[b]:
                    dt2w[64:128, 2 * syi, m] = wt2[:, 0, ky, kx_of[b][-1]]
                # solo round: rows 0-63 sx=+1
                if 1 in kx_of[b]:
                    dt2w[0:64, 2 * syi + 1, m] = wt2[:, 0, ky, kx_of[b][1]]
    shared['dt2w'] = dt2w.reshape(128, 24)

    in_maps = []
    for core in range(NCORES):
        m = dict(shared)
        cols = np.zeros((16, 2, 128, 128), F)
        for i in range(2):
            img = x[2 * core + i, 0]
            xp = np.zeros((258, 258), F)
            xp[1:257, 1:257] = img
            for ky in range(4):
                for kx in range(4):
                    cols[ky * 4 + kx, i] = xp[ky:ky + 256:2, kx:kx + 256:2][:128, :128]
        m['xcol'] = cols.reshape(16, 2 * 16384)
        in_maps.append(m)
    return in_maps, g


_NC_CACHE = {}


def kernel(**inputs):
    in_maps, g = _prep_inputs(inputs)
    if 'nc' not in _NC_CACHE:
        _NC_CACHE['nc'] = _build()
    nc = _NC_CACHE['nc']
    res = run_bass_kernel_spmd(nc, in_maps, list(range(NCORES)))
    results = res.results

    F = np.float32
    x_recon = np.zeros((16, 1, 256, 256), F)
    dt2b = float(g['dt2_b'][0])
    tot = 0.0
    idx2_all = []
    for core in range(NCORES):
        r = results[core]
        y = r['out_y'].reshape(2, 4, 128, 128)
        for i in range(2):
            for a in range(2):
                for b in range(2):
                    x_recon[2 * core + i, 0, a::2, b::2] = y[i, 2 * a + b]
        tot += float(r['out_zsq'].astype(np.float64).sum())
        tot -= 2.0 * float(r['out_smax'].astype(np.float64).sum())
        idx2_all.append(r['out_idx2'].T.reshape(-1))
    x_recon += F(dt2b)

    e_latent = tot / (16 * 4096 * 128)
    loss = F(0.25 * e_latent)

    counts = np.bincount(np.concatenate(idx2_all).astype(np.int64),
                         minlength=1024).astype(np.float64)
    avg = counts / (16 * 4096)
    perp = F(np.exp(-np.sum(avg * np.log(avg + 1e-10))))
    return loss, x_recon, perp


# revision 33
# speedup vs baseline: 1.0648x; 1.0047x over previous
"""BeamVQ (VQ-VAE fwd) Trainium2 kernel — 8-core batch-parallel.

Strategy: shard batch 16 -> 8 cores x 2 images. Convs as shift-matmuls on PE
(channels on partitions, PSUM accumulation). Training-mode BN via per-channel
sum/sumsq with a tiny in-kernel AllReduce per BN layer (8 total). VQ: PE fp32
distance scores s = z.e - |e|^2/2, DVE max/max_index top-8, gpsimd ap_gather
codebook lookup. ConvTranspose via output-parity decomposition. Loss /
perplexity partial sums are finished on the host.

Precision: encoder + pre-VQ + distance matmuls in true fp32 (top-1 gaps go
down to 5e-5; fp32r's ~1e-5 operand rounding flips argmins and corrupts the
reconstruction by O(1)). Decoder matmuls run in fp32r (4x faster PE streams;
weights/activations are rounded to f32r by their ACT producers as the BIR
verifier requires) giving recon absmax ~1.4e-3 on scale 3.7; set DEC_DT = f32
to get absmax ~6e-6 at +~550us.

Measured (8 trn2 cores via axon/PJRT): recon absmax 1.38e-3 (rel 3.8e-4),
loss rel 3.5e-7, perplexity rel 5.5e-6 vs the fp32 CPU reference.
Cost-model timeline estimate ~1.29 ms end-to-end (excl. AllReduce latency).
Filler matmuls in the BN/collective valleys keep the PE clock warm; dup-half
padded writes are split ACT(main)/DVE(shifted dup) to balance engines.
"""
import numpy as np

import concourse.bass as bass
import concourse.mybir as mybir
from concourse import bacc, tile
from concourse.bass_utils import run_bass_kernel_spmd

f32 = mybir.dt.float32
f32r = mybir.dt.float32r
u16 = mybir.dt.uint16
i16 = mybir.dt.int16

NCORES = 8
ENC_DT = f32    # z-path matmul dtype (must stay fp32)
DEC_DT = f32r   # decoder matmul dtype (operands produced as rounded f32r)

POS9 = [(ky, kx) for ky in range(3) for kx in range(3)]
# transposed-conv k4 s2 p1 parity taps: parity -> [(k, shift)]
TAPS = {0: [(1, 0), (3, 1)], 1: [(0, -1), (2, 0)]}

A = mybir.ActivationFunctionType


def _mm(nc, ps, lhsT, rhs, dt, start, stop):
    # operands must already carry dtype `dt` (f32r data must be produced
    # rounded — the BIR verifier rejects plain bitcasts of f32 data)
    nc.tensor.matmul(ps, lhsT, rhs, start=start, stop=stop)


def _conv3x3(nc, pp, src, wsl, Cin, Cout, dt, emit, name):
    """src: padded-66 AP [>=Cin, 4356]; wsl(pos) -> lhsT [Cin, Cout];
    emit(bank, ps) consumes psum [Cout, 512] for out rows 8*bank..8*bank+8."""
    v = src.rearrange("p (h w) -> p h w", w=66)
    for bank in range(8):
        y0 = 8 * bank
        ps = pp.tile([Cout, 512], f32, tag="ps", name=f"{name}_ps{bank}")
        for pos, (ky, kx) in enumerate(POS9):
            rhs = v[0:Cin, y0 + ky:y0 + ky + 8, kx:kx + 64]
            _mm(nc, ps[:], wsl(pos), rhs, dt, pos == 0, pos == 8)
        emit(bank, ps)


def _memset_rings66(nc, t):
    v = t.bitcast(mybir.dt.uint32).rearrange("p (h w) -> p h w", w=66)
    nc.gpsimd.memset(v[:, 0:66:65, :], 0)
    nc.gpsimd.memset(v[:, :, 0:1], 0)
    nc.gpsimd.memset(v[:, :, 65:66], 0)


def _memset_rings130(nc, t):
    v = t.bitcast(mybir.dt.uint32).rearrange("p (h w) -> p h w", w=130)
    nc.gpsimd.memset(v[:, 0:130:129, :], 0)
    nc.gpsimd.memset(v[:, :, 0:1], 0)
    nc.gpsimd.memset(v[:, :, 128:130], 0)


def _build(with_cc=True):
    nc = bacc.Bacc("TRN2", target_bir_lowering=False, debug=False,
                   enable_asserts=False,
                   num_devices=NCORES if with_cc else 1)

    def din(name, shape, dt=f32):
        return nc.dram_tensor(name, shape, dt, kind="ExternalInput")

    def dout(name, shape, dt=f32):
        return nc.dram_tensor(name, shape, dt, kind="ExternalOutput")

    xcol_d = din("xcol", [16, 2 * 16384])
    w1_d = din("w1", [16, 128])
    b1_d = din("b1", [128, 1])
    w2p_d = din("w2p", [128, 8 * 128])
    b2_d = din("b2", [128, 1])
    w3_d = din("w3", [128, 9 * 128])
    b3_d = din("b3", [128, 1])
    ewa_d = din("ewa", [128, 2 * 9 * 32])
    ewb_d = din("ewb", [32, 2 * 128])
    eg1_d = din("eg1", [32, 2])
    eb1_d = din("eb1", [32, 2])
    eg2_d = din("eg2", [128, 2])
    eb2_d = din("eb2", [128, 2])
    prew_d = din("prew", [128, 128])
    preb_d = din("preb", [128, 1])
    embT_d = din("embT", [128, 1024])
    e2neg_d = din("e2neg", [1, 1024])
    dw1_d = din("dw1", [128, 9 * 128])
    db1_d = din("db1", [128, 1])
    dwa_d = din("dwa", [128, 2 * 9 * 32])
    dwb_d = din("dwb", [32, 2 * 128])
    dg1_d = din("dg1", [32, 2])
    db1r_d = din("db1r", [32, 2])
    dg2_d = din("dg2", [128, 2])
    db2r_d = din("db2r", [128, 2])
    dt1w_d = din("dt1w", [128, 16 * 128])
    dt1b_d = din("dt1b", [128, 1])
    dt2w_d = din("dt2w", [128, 24])

    outy_d = dout("out_y", [2, 4, 16384])
    osmax_d = dout("out_smax", [128, 64])
    oidx2_d = dout("out_idx2", [128, 64], u16)
    ozsq_d = dout("out_zsq", [128, 1])

    with tile.TileContext(nc) as tc:
        with tc.tile_pool(name="wpool", bufs=1) as wp, \
             tc.tile_pool(name="main", bufs=1) as mp, \
             tc.tile_pool(name="small", bufs=2) as sp, \
             tc.tile_pool(name="dram", bufs=1, space="DRAM") as dr:

            w1 = wp.tile_from(w1_d[:])
            b1 = wp.tile_from(b1_d[:])
            w2p = wp.tile_from(w2p_d[:])
            b2 = wp.tile_from(b2_d[:])
            w3 = wp.tile_from(w3_d[:])
            b3 = wp.tile_from(b3_d[:])
            ewa = wp.tile_from(ewa_d[:])
            ewb = wp.tile_from(ewb_d[:])
            eg1 = wp.tile_from(eg1_d[:])
            eb1 = wp.tile_from(eb1_d[:])
            eg2 = wp.tile_from(eg2_d[:])
            eb2 = wp.tile_from(eb2_d[:])
            prew = wp.tile_from(prew_d[:])
            preb = wp.tile_from(preb_d[:])
            embT = wp.tile_from(embT_d[:])
            dw1 = wp.tile_from(dw1_d[:])
            db1 = wp.tile_from(db1_d[:])
            dwa = wp.tile_from(dwa_d[:])
            dwb = wp.tile_from(dwb_d[:])
            dg1 = wp.tile_from(dg1_d[:])
            db1r = wp.tile_from(db1r_d[:])
            dg2 = wp.tile_from(dg2_d[:])
            db2r = wp.tile_from(db2r_d[:])

            cc_i32 = dr.tile([32, 2], f32, name="cc_i32")
            cc_o32 = dr.tile([32, 2], f32, name="cc_o32")
            cc_i128 = dr.tile([128, 2], f32, name="cc_i128")
            cc_o128 = dr.tile([128, 2], f32, name="cc_o128")
            idx0_scr = dr.tile([8192], u16, name="idx0_scr")

            eps_t = sp.tile([128, 1], f32, tag="eps", bufs=1, name="eps_t")
            nc.gpsimd.memset(eps_t[:], 1e-5)
            coll_i0 = sp.tile([128, 64], u16, tag="ci0", bufs=1, name="coll_i0")
            coll_i2 = sp.tile([128, 64], u16, tag="ci2", bufs=1, name="coll_i2")
            coll_sm = sp.tile([128, 64], f32, tag="csm", bufs=1, name="coll_sm")

            def bn_cc(raw, C, g_ap, b_ap, cci, cco, name, pp=None):
                """raw: SBUF AP [C, 8192] of raw conv outputs (both images).
                Returns (a, c) coefficient APs [C, 1]."""
                st6 = sp.tile([C, 96], f32, tag=f"st6_{C}", name=f"{name}_st6")
                for i in range(16):
                    nc.vector.bn_stats(st6[:, 6 * i:6 * i + 6],
                                       raw[:, 512 * i:512 * (i + 1)])
                mv = sp.tile([C, 2], f32, tag=f"mv_{C}", name=f"{name}_mv")
                nc.vector.bn_aggr(mv[:], st6[:])
                m2 = sp.tile([C, 1], f32, tag=f"m2_{C}", name=f"{name}_m2")
                nc.vector.tensor_tensor(m2[:], mv[:, 0:1], mv[:, 0:1],
                                        op=mybir.AluOpType.mult)
                ex2 = sp.tile([C, 1], f32, tag=f"ex2_{C}", name=f"{name}_ex2")
                nc.vector.tensor_tensor(ex2[:], mv[:, 1:2], m2[:],
                                        op=mybir.AluOpType.add)
                csb = sp.tile([C, 2], f32, tag=f"csb_{C}", name=f"{name}_csb")
                nc.vector.tensor_scalar_mul(csb[:, 0:1], mv[:, 0:1], 8192.0)
                nc.vector.tensor_scalar_mul(csb[:, 1:2], ex2[:], 8192.0)
                nc.sync.dma_start(cci[:], csb[:])
                if with_cc:
                    nc.gpsimd.collective_compute(
                        "AllReduce", mybir.AluOpType.add,
                        replica_groups=[list(range(NCORES))],
                        ins=[cci.opt()], outs=[cco.opt()])
                else:
                    nc.sync.dma_start(cco[:], cci[:])
                if pp is not None:
                    fill = pp.tile([128, 512], f32, tag="fill", bufs=1,
                                   name=f"{name}_fill")
                    for k in range(24):
                        nc.tensor.matmul(fill[:], embT[:, 0:128],
                                         embT[:, 0:512], start=True,
                                         stop=True)
                gsb = sp.tile([C, 2], f32, tag=f"gsb_{C}", name=f"{name}_gsb")
                nc.sync.dma_start(gsb[:], cco[:])
                gm = sp.tile([C, 1], f32, tag=f"gm_{C}", name=f"{name}_gm")
                nc.vector.tensor_scalar_mul(gm[:], gsb[:, 0:1], 1.0 / 65536.0)
                gex2 = sp.tile([C, 1], f32, tag=f"gex2_{C}", name=f"{name}_gex2")
                nc.vector.tensor_scalar_mul(gex2[:], gsb[:, 1:2], 1.0 / 65536.0)
                gm2 = sp.tile([C, 1], f32, tag=f"gm2_{C}", name=f"{name}_gm2")
                nc.vector.tensor_tensor(gm2[:], gm[:], gm[:],
                                        op=mybir.AluOpType.mult)
                gvar = sp.tile([C, 1], f32, tag=f"gvar_{C}", name=f"{name}_gvar")
                nc.vector.tensor_tensor(gvar[:], gex2[:], gm2[:],
                                        op=mybir.AluOpType.subtract)
                sd = sp.tile([C, 1], f32, tag=f"sd_{C}", name=f"{name}_sd")
                nc.scalar.activation(sd[:], gvar[:], A.Sqrt, bias=eps_t[0:C, :])
                inv = sp.tile([C, 1], f32, tag=f"inv_{C}", name=f"{name}_inv")
                nc.vector.reciprocal(inv[:], sd[:])
                a_t = sp.tile([C, 1], f32, tag=f"a_{C}", name=f"{name}_a")
                nc.vector.tensor_tensor(a_t[:], g_ap, inv[:],
                                        op=mybir.AluOpType.mult)
                am = sp.tile([C, 1], f32, tag=f"am_{C}", name=f"{name}_am")
                nc.vector.tensor_tensor(am[:], a_t[:], gm[:],
                                        op=mybir.AluOpType.mult)
                c_t = sp.tile([C, 1], f32, tag=f"c_{C}", name=f"{name}_c")
                nc.vector.tensor_tensor(c_t[:], b_ap, am[:],
                                        op=mybir.AluOpType.subtract)
                return a_t[:], c_t[:]

            def res_stack(P, pp, xs, wa, wb, g1s, b1s, g2s, b2s, dt, cci32,
                          cco32, cci128, cco128, pre):
                """xs: list of 2 padded-66 x-state tile APs. Returns xs."""
                for blk in range(2):
                    xr = []
                    for i in range(2):
                        t = P.tile([128, 4356], dt, tag="xr", bufs=1,
                                   name=f"{pre}xr{blk}_{i}")
                        for q in range(4):
                            nc.scalar.activation(t[:, 1089 * q:1089 * (q + 1)],
                                                 xs[i][:, 1089 * q:1089 * (q + 1)],
                                                 A.Relu)
                        xr.append(t)
                    rawa = P.tile([32, 8192], f32, tag="rawa", bufs=1,
                                  name=f"{pre}rawa{blk}")

                    for i in range(2):
                        def emit_a(bank, ps, i=i):
                            off = 4096 * i + 512 * bank
                            nc.scalar.activation(rawa[:, off:off + 512], ps[:],
                                                 A.Copy)
                        _conv3x3(nc, pp, xr[i][:],
                                 lambda p: wa[:, (9 * blk + p) * 32:
                                              (9 * blk + p) * 32 + 32],
                                 128, 32, dt, emit_a, f"{pre}ca{blk}_{i}")
                    a1, c1 = bn_cc(rawa[:], 32, g1s[:, blk:blk + 1],
                                   b1s[:, blk:blk + 1], cci32, cco32,
                                   f"{pre}bn1_{blk}", pp=pp)
                    for q in range(16):
                        sl = slice(512 * q, 512 * (q + 1))
                        nc.scalar.activation(rawa[:, sl], rawa[:, sl], A.Relu,
                                             bias=c1, scale=a1)
                    rawb = P.tile([128, 8192], f32, tag="rawb", bufs=1,
                                  name=f"{pre}rawb{blk}")
                    for i in range(2):
                        for bank in range(8):
                            off = 4096 * i + 512 * bank
                            ps = pp.tile([128, 512], f32, tag="ps",
                                         name=f"{pre}cb{blk}_{i}_{bank}")
                            _mm(nc, ps[:],
                                wb[:, 128 * blk:128 * blk + 128],
                                rawa[0:32, off:off + 512], f32, True, True)
                            nc.scalar.activation(rawb[:, off:off + 512],
                                                 ps[:], A.Copy)
                    a2, c2 = bn_cc(rawb[:], 128, g2s[:, blk:blk + 1],
                                   b2s[:, blk:blk + 1], cci128, cco128,
                                   f"{pre}bn2_{blk}", pp=pp)
                    for q in range(16):
                        sl = slice(512 * q, 512 * (q + 1))
                        nc.scalar.activation(rawb[:, sl], rawb[:, sl],
                                             A.Identity, bias=c2, scale=a2)
                    for i in range(2):
                        xv = xs[i][:].rearrange("p (h w) -> p h w", w=66)
                        xint = xv[:, 1:65, 1:65]
                        rv = rawb[:, 4096 * i:4096 * (i + 1)].rearrange(
                            "p (h w) -> p h w", w=64)
                        nc.vector.tensor_tensor(xint, xint, rv,
                                                op=mybir.AluOpType.add)
                return xs

            # ============ P1: conv1 + conv2 + conv3 ============
            xst = []
            for i in range(2):
                t = mp.tile([128, 4356], f32, tag="xst", bufs=2,
                            name=f"xst{i}")
                xst.append(t)
            with tc.tile_pool(name="p1", bufs=1) as p1, \
                 tc.tile_pool(name="pp1", bufs=8, space="PSUM") as pp1:
                for i in range(2):
                    X1 = p1.tile([128, 16900], f32, tag="X1", bufs=1,
                                 name=f"X1_{i}")
                    _memset_rings130(nc, X1[:])
                    X1v = X1[:].rearrange("p (h w) -> p h w", w=130)
                    for c in range(32):
                        stg = p1.tile([16, 512], f32, tag="c1s", bufs=3,
                                      name=f"c1s_{i}_{c}")
                        nc.gpsimd.dma_start(
                            stg[:], xcol_d[:, 16384 * i + 512 * c:
                                           16384 * i + 512 * (c + 1)])
                        ps = pp1.tile([128, 512], f32, tag="ps",
                                      name=f"c1ps_{i}_{c}")
                        _mm(nc, ps[:], w1[:], stg[:], ENC_DT, True, True)
                        psv = ps[:].rearrange("p (a b) -> p a b", b=128)
                        nc.scalar.activation(
                            X1v[0:64, 1 + 4 * c:5 + 4 * c, 1:129],
                            psv[0:64], A.Relu, bias=b1[0:64, :])
                        nc.scalar.activation(
                            X1v[64:128, 1 + 4 * c:5 + 4 * c, 0:128],
                            psv[64:128], A.Relu, bias=b1[64:128, :])
                    c2o = p1.tile([128, 4356], f32, tag="c2o", bufs=1,
                                  name=f"c2o_{i}")
                    _memset_rings66(nc, c2o[:])
                    c2ov = c2o[:].rearrange("p (h w) -> p h w", w=66)
                    for bank in range(8):
                        y0 = 8 * bank
                        ps = pp1.tile([128, 512], f32, tag="ps",
                                      name=f"c2ps_{i}_{bank}")
                        for r in range(8):
                            ky, kxp = r // 2, r % 2
                            rhs = X1v[:, 2 * y0 + ky:2 * y0 + ky + 15:2,
                                      2 * kxp:2 * kxp + 127:2]
                            _mm(nc, ps[:], w2p[:, 128 * r:128 * (r + 1)],
                                rhs, ENC_DT, r == 0, r == 7)
                        psv = ps[:].rearrange("p (a b) -> p a b", b=64)
                        nc.scalar.activation(c2ov[:, 1 + y0:9 + y0, 1:65],
                                             psv, A.Relu, bias=b2[:])
                    _memset_rings66(nc, xst[i][:])
                    xiv = xst[i][:].rearrange("p (h w) -> p h w", w=66)

                    def emit3(bank, ps, xiv=xiv):
                        y0 = 8 * bank
                        psv = ps[:].rearrange("p (a b) -> p a b", b=64)
                        nc.scalar.activation(xiv[:, 1 + y0:9 + y0, 1:65],
                                             psv, A.Identity, bias=b3[:])
                    _conv3x3(nc, pp1, c2o[:],
                             lambda p: w3[:, 128 * p:128 * (p + 1)],
                             128, 128, ENC_DT, emit3, f"c3_{i}")

            # ============ P2a: encoder res stack ============
            with tc.tile_pool(name="p2a", bufs=1) as p2a, \
                 tc.tile_pool(name="pp2a", bufs=7, space="PSUM") as pp2a:
                res_stack(p2a, pp2a, xst, ewa, ewb, eg1, eb1, eg2, eb2,
                          ENC_DT, cc_i32, cc_o32, cc_i128, cc_o128, "e")
                efill = pp2a.tile([128, 512], f32, tag="fill", bufs=1,
                                  name="efill")
                for k in range(10):
                    nc.tensor.matmul(efill[:], embT[:, 0:128],
                                     embT[:, 0:512], start=True, stop=True)
                xrf = []
                for i in range(2):
                    t = mp.tile([128, 4356], f32, tag="xrf", bufs=2,
                                name=f"exrf{i}")
                    for q in range(4):
                        nc.scalar.activation(t[:, 1089 * q:1089 * (q + 1)],
                                             xst[i][:, 1089 * q:1089 * (q + 1)],
                                             A.Relu)
                    xrf.append(t)

            # ============ P2b: pre-VQ conv + VQ ============
            with tc.tile_pool(name="p2b", bufs=1) as p2b, \
                 tc.tile_pool(name="pp2b", bufs=2, space="PSUM") as pp2b:
                z = p2b.tile([128, 8192], f32, tag="z", bufs=1, name="z")
                for i in range(2):
                    xv = xrf[i][:].rearrange("p (h w) -> p h w", w=66)
                    for bank in range(8):
                        y0 = 8 * bank
                        ps = pp2b.tile([128, 512], f32, tag="pvps", bufs=2,
                                       name=f"pv_{i}_{bank}")
                        _mm(nc, ps[:], prew[:], xv[:, 1 + y0:9 + y0, 1:65],
                            ENC_DT, True, True)
                        off = 4096 * i + 512 * bank
                        nc.scalar.activation(z[:, off:off + 512], ps[:],
                                             A.Identity, bias=preb[:])
                q0T = p2b.tile([128, 8192], f32, tag="q0T", bufs=1, name="q0T")
                zsq = sp.tile([128, 1], f32, tag="zsq", bufs=1, name="zsq")
                nc.scalar.activation(q0T[:], z[:], A.Square, accum_out=zsq[:])
                nc.sync.dma_start(ozsq_d[:], zsq[:])

                e2neg_t = p2b.tile_from(e2neg_d[:])
                e2rep = p2b.tile([128, 1024], f32, tag="e2rep", bufs=1,
                                 name="e2rep")
                nc.gpsimd.partition_broadcast(e2rep[:], e2neg_t[:])

                for c in range(64):
                    ps = pp2b.tile([128, 1024], f32, tag="vqps", bufs=3,
                                   name=f"vqps{c}")
                    for h in range(2):
                        _mm(nc, ps[:, 512 * h:512 * (h + 1)],
                            z[:, 128 * c:128 * (c + 1)],
                            embT[:, 512 * h:512 * (h + 1)], ENC_DT,
                            True, True)
                    s_t = p2b.tile([128, 1024], f32, tag="s", bufs=4,
                                   name=f"s{c}")
                    nc.scalar.activation(s_t[:, 0:512], ps[:, 0:512], A.Copy)
                    nc.scalar.activation(s_t[:, 512:1024], ps[:, 512:1024],
                                         A.Copy)
                    nc.gpsimd.tensor_tensor(s_t[:], s_t[:], e2rep[:],
                                            op=mybir.AluOpType.add)
                    mx = p2b.tile([128, 8], f32, tag="mx", bufs=3,
                                  name=f"mx{c}")
                    ix = p2b.tile([128, 8], u16, tag="ix", bufs=3,
                                  name=f"ix{c}")
                    nc.vector.max(mx[:], s_t[:])
                    nc.vector.max_index(ix[:], mx[:], s_t[:])
                    nc.gpsimd.tensor_copy(coll_i0[:, c:c + 1], ix[:, 0:1])
                    nc.gpsimd.tensor_copy(coll_i2[:, c:c + 1], ix[:, 2:3])
                    if c % 8 == 7:
                        g0 = c - 7
                        scr_v = idx0_scr[:].rearrange("(p c) -> p c", c=64)
                        nc.sync.dma_start(scr_v[:, g0:g0 + 8],
                                          coll_i0[:, g0:g0 + 8])
                    nc.scalar.activation(coll_sm[:, c:c + 1], mx[:, 0:1],
                                         A.Copy)

                vqfill = pp2b.tile([128, 512], f32, tag="pvps",
                                   name="vqfill")
                for k in range(20):
                    nc.tensor.matmul(vqfill[:], embT[:, 0:128],
                                     embT[:, 0:512], start=True, stop=True)
                nc.sync.dma_start(oidx2_d[:], coll_i2[:])
                nc.sync.dma_start(osmax_d[:], coll_sm[:])
                wrapped = p2b.tile([128, 512], u16, tag="wrapped", bufs=1,
                                   name="wrapped")
                # slots 64*vg..64*vg+64 <=> chunks 8*vg..8*vg+8 (vecs 1024*vg..)
                srcv = idx0_scr[:].rearrange("(q r c) -> r c q", q=8, r=16,
                                             c=64)
                for g in range(8):
                    dst = wrapped[16 * g:16 * (g + 1), :].rearrange(
                        "r (c q) -> r c q", q=8)
                    nc.sync.dma_start(dst, srcv)
                nc.gpsimd.ap_gather(q0T[:], embT[:],
                                    wrapped[:].bitcast(i16), channels=128,
                                    num_elems=1024, d=1, num_idxs=8192)

                # quantized -> decoder input (padded, reuse xrf slots)
                q0pad = []
                for i in range(2):
                    t = mp.tile([128, 4356], DEC_DT, tag="xrf", bufs=2,
                                name=f"q0pad{i}")
                    _memset_rings66(nc, t[:])
                    tv = t[:].rearrange("p (h w) -> p h w", w=66)
                    qv = q0T[:, 4096 * i:4096 * (i + 1)].rearrange(
                        "p (h w) -> p h w", w=64)
                    for q in range(4):
                        nc.scalar.activation(
                            tv[:, 1 + 16 * q:1 + 16 * (q + 1), 1:65],
                            qv[:, 16 * q:16 * (q + 1), :], A.Copy)
                    q0pad.append(t)

            # ============ P3a: d_w1 conv + decoder res stack ============
            with tc.tile_pool(name="p3a", bufs=1) as p3a, \
                 tc.tile_pool(name="pp3a", bufs=7, space="PSUM") as pp3a:
                if DEC_DT != f32:
                    dw1r = p3a.tile([128, 9 * 128], DEC_DT, tag="dw1r",
                                    bufs=1, name="dw1r")
                    nc.scalar.activation(dw1r[:], dw1[:], A.Copy)
                    dwar = p3a.tile([128, 2 * 9 * 32], DEC_DT, tag="dwar",
                                    bufs=1, name="dwar")
                    nc.scalar.activation(dwar[:], dwa[:], A.Copy)
                else:
                    dw1r, dwar = dw1, dwa
                for i in range(2):
                    _memset_rings66(nc, xst[i][:])
                    yv = xst[i][:].rearrange("p (h w) -> p h w", w=66)

                    def emitd(bank, ps, yv=yv):
                        y0 = 8 * bank
                        psv = ps[:].rearrange("p (a b) -> p a b", b=64)
                        nc.scalar.activation(yv[:, 1 + y0:9 + y0, 1:65],
                                             psv, A.Identity, bias=db1[:])
                    _conv3x3(nc, pp3a, q0pad[i][:],
                             lambda p: dw1r[:, 128 * p:128 * (p + 1)],
                             128, 128, DEC_DT, emitd, f"dw1_{i}")
                res_stack(p3a, pp3a, xst, dwar, dwb, dg1, db1r, dg2, db2r,
                          DEC_DT, cc_i32, cc_o32, cc_i128, cc_o128, "d")
                dfill = pp3a.tile([128, 512], f32, tag="fill", bufs=1,
                                  name="dfill")
                for k in range(36):
                    nc.tensor.matmul(dfill[:], embT[:, 0:128],
                                     embT[:, 0:512], start=True, stop=True)
                yrf = []
                for i in range(2):
                    t = mp.tile([128, 4356], DEC_DT, tag="xrf", bufs=2,
                                name=f"dxrf{i}")
                    for q in range(4):
                        nc.scalar.activation(t[:, 1089 * q:1089 * (q + 1)],
                                             xst[i][:, 1089 * q:1089 * (q + 1)],
                                             A.Relu)
                    yrf.append(t)

            # ============ P3b: dt1 + dt2 ============
            with tc.tile_pool(name="p3b", bufs=1) as p3b, \
                 tc.tile_pool(name="pp3b", bufs=8, space="PSUM") as pp3b:
                dt1w = p3b.tile_from(dt1w_d[:])
                dt1b = p3b.tile_from(dt1b_d[:])
                dt2w = p3b.tile_from(dt2w_d[:])
                if DEC_DT != f32:
                    dt1wr = p3b.tile([128, 16 * 128], DEC_DT, tag="dt1wr",
                                     bufs=1, name="dt1wr")
                    nc.scalar.activation(dt1wr[:], dt1w[:], A.Copy)
                    dt2wr = p3b.tile([128, 24], DEC_DT, tag="dt2wr",
                                     bufs=1, name="dt2wr")
                    nc.scalar.activation(dt2wr[:], dt2w[:], A.Copy)
                else:
                    dt1wr, dt2wr = dt1w, dt2w
                for i in range(2):
                    X2 = p3b.tile([128, 16900], DEC_DT, tag="X2", bufs=1,
                                  name=f"X2_{i}")
                    _memset_rings130(nc, X2[:])
                    X2v = X2[:].rearrange("p (h w) -> p h w", w=130)
                    yv = yrf[i][:].rearrange("p (h w) -> p h w", w=66)
                    for a in range(2):
                        for b in range(2):
                            for bank in range(8):
                                u0 = 8 * bank
                                ps = pp3b.tile([128, 512], f32, tag="ps",
                                               name=f"t1_{i}_{a}{b}_{bank}")
                                k = 0
                                for t_i, (ky, sy) in enumerate(TAPS[a]):
                                    for s_i, (kx, sx) in enumerate(TAPS[b]):
                                        idx = ((a * 2 + b) * 2 + t_i) * 2 + s_i
                                        rhs = yv[:, 1 + u0 - sy:9 + u0 - sy,
                                                 1 - sx:65 - sx]
                                        _mm(nc, ps[:],
                                            dt1wr[:, 128 * idx:128 * (idx + 1)],
                                            rhs, DEC_DT, k == 0, k == 3)
                                        k += 1
                                psv = ps[:].rearrange("p (a b) -> p a b", b=64)
                                r0 = 1 + 2 * u0 + a
                                nc.scalar.activation(
                                    X2v[0:64, r0:r0 + 16:2,
                                        1 + b:129 + b:2],
                                    psv[0:64], A.Relu, bias=dt1b[0:64, :])
                                nc.vector.tensor_scalar(
                                    X2v[64:128, r0:r0 + 16:2, b:128 + b:2],
                                    psv[64:128], dt1b[64:128, :], 0.0,
                                    op0=mybir.AluOpType.add,
                                    op1=mybir.AluOpType.max)
                    for ch in range(32):
                        u0 = 4 * ch
                        ps2 = pp3b.tile([4, 512], f32, tag="ps",
                                        name=f"t2_{i}_{ch}")
                        for syi, sy in enumerate((-1, 0, 1)):
                            r0 = 1 + u0 - sy
                            rhs = X2v[0:128, r0:r0 + 4, 1:129]
                            _mm(nc, ps2[:], dt2wr[:, 8 * syi:8 * syi + 4],
                                rhs, DEC_DT, syi == 0, False)
                            rhs2 = X2v[0:64, r0:r0 + 4, 0:128]
                            _mm(nc, ps2[:],
                                dt2wr[0:64, 8 * syi + 4:8 * syi + 8],
                                rhs2, DEC_DT, False, syi == 2)
                        yst = p3b.tile([4, 512], f32, tag="yst", bufs=3,
                                       name=f"yst_{i}_{ch}")
                        nc.vector.tensor_copy(yst[:], ps2[:])
                        nc.sync.dma_start(
                            outy_d[i:i + 1, :, 512 * ch:512 * (ch + 1)],
                            yst[:])
    nc.compile()
    return nc


def _prep_inputs(inp):
    F = np.float32
    g = {k: np.ascontiguousarray(np.asarray(v), dtype=F) for k, v in inp.items()
         if k != 'emb'}
    g['emb'] = np.ascontiguousarray(np.asarray(inp['emb']), dtype=F)
    x = g['x']

    shared = {}
    shared['w1'] = np.concatenate([g['p_w1'].reshape(64, 16).T] * 2, axis=1)
    shared['b1'] = np.concatenate([g['p_b1']] * 2)[:, None]

    w2 = g['p_w2']  # [128, 64, 4, 4]
    w2p = np.zeros((128, 8, 128), F)
    for r in range(8):
        ky, kxp = r // 2, r % 2
        w2p[0:64, r] = w2[:, :, ky, 2 * kxp].T
        w2p[64:128, r] = w2[:, :, ky, 2 * kxp + 1].T
    shared['w2p'] = w2p.reshape(128, 1024)
    shared['b2'] = g['p_b2'][:, None]

    def conv9(w):  # [Cout, Cin, 3, 3] -> [Cin, 9*Cout]
        Cout, Cin = w.shape[0], w.shape[1]
        out = np.zeros((Cin, 9, Cout), F)
        for p, (ky, kx) in enumerate(POS9):
            out[:, p] = w[:, :, ky, kx].T
        return out.reshape(Cin, 9 * Cout)

    shared['w3'] = conv9(g['p_w3'])
    shared['b3'] = g['p_b3'][:, None]

    def resw(wa, wb):
        ewa = np.zeros((128, 2, 9, 32), F)
        for blk in range(2):
            ewa[:, blk] = conv9(wa[blk]).reshape(128, 9, 32)
        ewb = np.zeros((32, 2, 128), F)
        for blk in range(2):
            ewb[:, blk] = wb[blk][:, :, 0, 0].T
        return ewa.reshape(128, 576), ewb.reshape(32, 256)

    shared['ewa'], shared['ewb'] = resw(g['pr_wa'], g['pr_wb'])
    shared['eg1'] = g['pr_g1'].T.copy()
    shared['eb1'] = g['pr_b1'].T.copy()
    shared['eg2'] = g['pr_g2'].T.copy()
    shared['eb2'] = g['pr_b2'].T.copy()
    shared['prew'] = g['pre_w'][:, :, 0, 0].T.copy()
    shared['preb'] = g['pre_b'][:, None]
    emb = g['emb']
    shared['embT'] = emb.T.copy()
    shared['e2neg'] = (-0.5 * (emb.astype(F) ** 2).sum(1))[None, :].astype(F)
    shared['dw1'] = conv9(g['d_w1'])
    shared['db1'] = g['d_b1'][:, None]
    shared['dwa'], shared['dwb'] = resw(g['dr_wa'], g['dr_wb'])
    shared['dg1'] = g['dr_g1'].T.copy()
    shared['db1r'] = g['dr_b1'].T.copy()
    shared['dg2'] = g['dr_g2'].T.copy()
    shared['db2r'] = g['dr_b2'].T.copy()

    wt1 = g['dt1_w']  # [128, 64, 4, 4] (Cin, Cout, kh, kw)
    dt1w = np.zeros((128, 16, 128), F)
    for a in range(2):
        for b in range(2):
            for t_i, (ky, sy) in enumerate(TAPS[a]):
                for s_i, (kx, sx) in enumerate(TAPS[b]):
                    idx = ((a * 2 + b) * 2 + t_i) * 2 + s_i
                    dt1w[:, idx, 0:64] = wt1[:, :, ky, kx]
                    dt1w[:, idx, 64:128] = wt1[:, :, ky, kx]
    shared['dt1w'] = dt1w.reshape(128, 2048)
    shared['dt1b'] = np.concatenate([g['dt1_b']] * 2)[:, None]

    wt2 = g['dt2_w']  # [64, 1, 4, 4]
    ky_of = {0: {0: 1, 1: 3}, 1: {-1: 0, 0: 2}}  # [a][sy] -> ky
    kx_of = {0: {0: 1, 1: 3}, 1: {-1: 0, 0: 2}}
    dt2w = np.zeros((128, 6, 4), F)
    for syi, sy in enumerate((-1, 0, 1)):
        for a in range(2):
            if sy not in ky_of[a]:
                continue
            ky = ky_of[a][sy]
            for b in range(2):
                m = 2 * a + b
                # pair round: rows 0-63 sx=0, rows 64-127 sx=-1
                if 0 in kx_of[b]:
                    dt2w[0:64, 2 * syi, m] = wt2[:, 0, ky, kx_of[b][0]]
                if -1 in kx_of[b]:
                    dt2w[64:128, 2 * syi, m] = wt2[:, 0, ky, kx_of[b][-1]]
                # solo round: rows 0-63 sx=+1
                if 1 in kx_of[b]:
                    dt2w[0:64, 2 * syi + 1, m] = wt2[:, 0, ky, kx_of[b][1]]
    shared['dt2w'] = dt2w.reshape(128, 24)

    in_maps = []
    for core in range(NCORES):
        m = dict(shared)
        cols = np.zeros((16, 2, 128, 128), F)
        for i in range(2):
            img = x[2 * core + i, 0]
            xp = np.zeros((258, 258), F)
            xp[1:257, 1:257] = img
            for ky in range(4):
                for kx in range(4):
                    cols[ky * 4 + kx, i] = xp[ky:ky + 256:2, kx:kx + 256:2][:128, :128]
        m['xcol'] = cols.reshape(16, 2 * 16384)
        in_maps.append(m)
    return in_maps, g


_NC_CACHE = {}


def kernel(**inputs):
    in_maps, g = _prep_inputs(inputs)
    if 'nc' not in _NC_CACHE:
        _NC_CACHE['nc'] = _build()
    nc = _NC_CACHE['nc']
    res = run_bass_kernel_spmd(nc, in_maps, list(range(NCORES)))
    results = res.results

    F = np.float32
    x_recon = np.zeros((16, 1, 256, 256), F)
    dt2b = float(g['dt2_b'][0])
    tot = 0.0
    idx2_all = []
    for core in range(NCORES):
        r = results[core]
        y = r['out_y'].reshape(2, 4, 128, 128)
        for i in range(2):
            for a in range(2):
                for b in range(2):
                    x_recon[2 * core + i, 0, a::2, b::2] = y[i, 2 * a + b]
        tot += float(r['out_zsq'].astype(np.float64).sum())
        tot -= 2.0 * float(r['out_smax'].astype(np.float64).sum())
        idx2_all.append(r['out_idx2'].T.reshape(-1))
    x_recon += F(dt2b)

    e_latent = tot / (16 * 4096 * 128)
    loss = F(0.25 * e_latent)

    counts = np.bincount(np.concatenate(idx2_all).astype(np.int64),
                         minlength=1024).astype(np.float64)
    avg = counts / (16 * 4096)
    perp = F(np.exp(-np.sum(avg * np.log(avg + 1e-10))))
    return loss, x_recon, perp
